# revision 20
# baseline (speedup 1.0000x reference)
"""Trainium2 Bass kernel for DigitConvolutionalModel.

Pipeline (per core, pure data-parallel over batch):
  x [8192, 784] --DMA--> SBUF batch-major --PE transpose--> feature-major tiles
  conv 3x3 as banded block-matmuls on PE -> relu -> fc1 (matmul) -> relu
  -> fc2 (matmul) + bias -> DMA out.

All activations live feature-major ([features, batch]) so the PE can contract
over the partition dim. The 3x3 conv is expressed as 13 small banded matmuls
per 512-batch tile using three constant band matrices built on the host from
conv_w (shift-invariant across 4-image-row blocks).
"""

import numpy as np
from contextlib import ExitStack

N_CORES = 8
B_FULL = 65536
B_CORE = B_FULL // N_CORES  # 8192
BT = 512                    # batch tile (matmul moving free dim)
NT = B_CORE // BT           # 16

_cache = {}


def _build_module(b_core=B_CORE, n_cores=N_CORES):
    import concourse.bass as bass
    import concourse.tile as tile
    from concourse import bacc, mybir

    f32 = mybir.dt.float32
    f32r = mybir.dt.float32r
    AF = mybir.ActivationFunctionType
    nt = b_core // BT

    nc = bacc.Bacc("TRN2", target_bir_lowering=False, debug=False,
                   num_devices=n_cores)

    x_d = nc.dram_tensor("x", [b_core, 784], f32, kind="ExternalInput").ap()
    kA_d = nc.dram_tensor("kA", [112, 104], f32, kind="ExternalInput").ap()
    kB_d = nc.dram_tensor("kB", [56, 104], f32, kind="ExternalInput").ap()
    kC_d = nc.dram_tensor("kC", [112, 52], f32, kind="ExternalInput").ap()
    w1_d = nc.dram_tensor("w1", [676, 128], f32, kind="ExternalInput").ap()
    b1_d = nc.dram_tensor("b1", [128, 1], f32, kind="ExternalInput").ap()
    w2_d = nc.dram_tensor("w2", [128, 10], f32, kind="ExternalInput").ap()
    b2_d = nc.dram_tensor("b2", [10, 1], f32, kind="ExternalInput").ap()
    id_d = nc.dram_tensor("iden", [128, 128], f32, kind="ExternalInput").ap()
    y_d = nc.dram_tensor("y", [10, b_core], f32, kind="ExternalOutput").ap()

    with tile.TileContext(nc) as tc, ExitStack() as ctx:
        const = ctx.enter_context(tc.tile_pool(name="const", bufs=1))
        xbm_p = ctx.enter_context(tc.tile_pool(name="xbm", bufs=4))
        xfm_p = ctx.enter_context(tc.tile_pool(name="xfm", bufs=21))
        h_p = ctx.enter_context(tc.tile_pool(name="h", bufs=14))
        h1_p = ctx.enter_context(tc.tile_pool(name="h1", bufs=2))
        o_p = ctx.enter_context(tc.tile_pool(name="osb", bufs=2))
        tp_ps = ctx.enter_context(tc.tile_pool(name="tp_ps", bufs=2, space="PSUM"))
        cv_ps = ctx.enter_context(tc.tile_pool(name="cv_ps", bufs=2, space="PSUM"))
        f1_ps = ctx.enter_context(tc.tile_pool(name="f1_ps", bufs=2, space="PSUM"))
        f2_ps = ctx.enter_context(tc.tile_pool(name="f2_ps", bufs=2, space="PSUM"))

        iden = const.tile([128, 128], f32, name="iden")
        nc.sync.dma_start(iden[:], id_d)

        def load_f32r(name, shape, src):
            stg = const.tile(shape, f32, tag=f"{name}_stg", name=f"{name}_stg")
            nc.sync.dma_start(stg[:], src)
            t = const.tile(shape, f32r, tag=name, name=name)
            nc.vector.tensor_copy(t[:], stg[:])
            return t

        kA = load_f32r("kA", [112, 104], kA_d)
        kB = load_f32r("kB", [56, 104], kB_d)
        kC = load_f32r("kC", [112, 52], kC_d)
        w1 = []
        offs = 0
        for b in range(7):
            m = 104 if b < 6 else 52
            w1.append(load_f32r(f"w1_{b}", [m, 128], w1_d[offs:offs + m, :]))
            offs += m
        w2 = load_f32r("w2", [128, 10], w2_d)
        b1 = const.tile([128, 1], f32, name="b1")
        nc.sync.dma_start(b1[:], b1_d)
        b2 = const.tile([10, 1], f32, name="b2")
        nc.sync.dma_start(b2[:], b2_d)

        for it in range(nt):
            # ---- load one batch tile, batch-major [128, 4, 784]
            xbm = xbm_p.tile([128, 4, 784], f32, name="xbm", tag="xbm")
            src = x_d[it * BT:(it + 1) * BT, :].rearrange("(c p) d -> p c d", p=128)
            (nc.sync if it % 2 == 0 else nc.scalar).dma_start(xbm[:], src)

            # ---- transpose to feature-major tiles xfm[t] = x.T rows 112t..112t+111
            xfm = []
            for t in range(7):
                tp = tp_ps.tile([112, BT], f32, name="tp", tag="tp")
                for c in range(4):
                    nc.tensor.transpose(tp[:, c * 128:(c + 1) * 128],
                                        xbm[:, c, 112 * t:112 * t + 112],
                                        iden[:])
                xf = xfm_p.tile([112, BT], f32r, tag="xfm", name=f"xfm{t}")
                if t % 2 == 0:
                    nc.vector.tensor_copy(xf[:], tp[:])
                else:
                    nc.scalar.copy(xf[:], tp[:])
                xfm.append(xf)

            # ---- conv as banded matmuls, relu into h blocks
            hs = []
            for b in range(6):
                cv = cv_ps.tile([104, BT], f32, name="cv", tag="cv")
                nc.tensor.matmul(cv[:], kA[:], xfm[b][:], start=True, stop=False)
                nc.tensor.matmul(cv[:], kB[:], xfm[b + 1][0:56, :],
                                 start=False, stop=True)
                h = h_p.tile([104, BT], f32r, tag="h", name=f"h{b}")
                if b % 2 == 0:
                    nc.vector.tensor_scalar_max(h[:], cv[:], 0.0)
                else:
                    nc.scalar.activation(h[:], cv[:], AF.Relu)
                hs.append(h)
            cv = cv_ps.tile([52, BT], f32, name="cv6", tag="cv")
            nc.tensor.matmul(cv[:], kC[:], xfm[6][:], start=True, stop=True)
            h = h_p.tile([52, BT], f32r, tag="h", name="h6")
            nc.vector.tensor_scalar_max(h[:], cv[:], 0.0)
            hs.append(h)

            # ---- fc1: accumulate 7 chunks, relu + bias
            f1 = f1_ps.tile([128, BT], f32, name="f1", tag="f1")
            for b in range(7):
                nc.tensor.matmul(f1[:], w1[b][:], hs[b][:],
                                 start=(b == 0), stop=(b == 6))
            h1 = h1_p.tile([128, BT], f32r, name="h1", tag="h1")
            nc.scalar.activation(h1[:], f1[:], AF.Relu, bias=b1[:])

            # ---- fc2 + bias
            f2 = f2_ps.tile([10, BT], f32, name="f2", tag="f2")
            nc.tensor.matmul(f2[:], w2[:], h1[:], start=True, stop=True)
            osb = o_p.tile([10, BT], f32, name="osb", tag="osb")
            nc.scalar.activation(osb[:], f2[:], AF.Identity, bias=b2[:])

            # ---- store (feature-major; host transposes)
            nc.sync.dma_start(y_d[:, it * BT:(it + 1) * BT], osb[:])

    nc.compile()
    return nc


def _host_prep(inputs):
    x = np.ascontiguousarray(np.asarray(inputs["x"], dtype=np.float32))
    w = np.asarray(inputs["conv_w"], dtype=np.float32)
    fc1_w = np.asarray(inputs["fc1_w"], dtype=np.float32)
    fc1_b = np.asarray(inputs["fc1_b"], dtype=np.float32)
    fc2_w = np.asarray(inputs["fc2_w"], dtype=np.float32)
    fc2_b = np.asarray(inputs["fc2_b"], dtype=np.float32)

    kA = np.zeros((112, 104), np.float32)
    kB = np.zeros((56, 104), np.float32)
    kC = np.zeros((112, 52), np.float32)
    for oi in range(4):
        for oj in range(26):
            m = oi * 26 + oj
            for di in range(3):
                for dj in range(3):
                    ri, ci = oi + di, oj + dj
                    if ri < 4:
                        kA[ri * 28 + ci, m] = w[di, dj]
                    else:
                        kB[(ri - 4) * 28 + ci, m] = w[di, dj]
    for oi in range(2):
        for oj in range(26):
            m = oi * 26 + oj
            for di in range(3):
                for dj in range(3):
                    kC[(oi + di) * 28 + (oj + dj), m] = w[di, dj]

    consts = {
        "kA": kA,
        "kB": kB,
        "kC": kC,
        "w1": np.ascontiguousarray(fc1_w.T),
        "b1": np.ascontiguousarray(fc1_b.reshape(128, 1)),
        "w2": np.ascontiguousarray(fc2_w.T),
        "b2": np.ascontiguousarray(fc2_b.reshape(10, 1)),
        "iden": np.eye(128, dtype=np.float32),
    }
    in_maps = []
    for c in range(N_CORES):
        m = {"x": x[c * B_CORE:(c + 1) * B_CORE]}
        m.update(consts)
        in_maps.append(m)
    return in_maps


GBT = 2048                  # batch rows per DMA-transpose group (4 tiles)


def _build_module_v4(b_core=B_CORE, n_cores=N_CORES):
    import concourse.bass as bass
    import concourse.tile as tile
    from concourse import bacc, mybir

    f32 = mybir.dt.float32
    f32r = mybir.dt.float32r
    bf16 = mybir.dt.bfloat16
    AF = mybir.ActivationFunctionType
    nt = b_core // BT

    nc = bacc.Bacc("TRN2", target_bir_lowering=False, debug=False,
                   num_devices=n_cores)

    x_d = nc.dram_tensor("x", [b_core, 784], bf16, kind="ExternalInput").ap()
    kA_d = nc.dram_tensor("kA", [112, 104], bf16, kind="ExternalInput").ap()
    kB_d = nc.dram_tensor("kB", [56, 104], bf16, kind="ExternalInput").ap()
    kC_d = nc.dram_tensor("kC", [112, 52], bf16, kind="ExternalInput").ap()
    id_d = nc.dram_tensor("iden", [128, 128], bf16, kind="ExternalInput").ap()
    w1_d = nc.dram_tensor("w1", [676, 128], f32, kind="ExternalInput").ap()
    b1_d = nc.dram_tensor("b1", [128, 1], f32, kind="ExternalInput").ap()
    w2_d = nc.dram_tensor("w2", [128, 10], f32, kind="ExternalInput").ap()
    b2_d = nc.dram_tensor("b2", [10, 1], f32, kind="ExternalInput").ap()
    y_d = nc.dram_tensor("y", [10, b_core], f32, kind="ExternalOutput").ap()

    with tile.TileContext(nc) as tc, ExitStack() as ctx:
        const = ctx.enter_context(tc.tile_pool(name="const", bufs=1))
        xbm_p = ctx.enter_context(tc.tile_pool(name="xbm", bufs=4))
        xfm_p = ctx.enter_context(tc.tile_pool(name="xfm", bufs=21))
        h_p = ctx.enter_context(tc.tile_pool(name="h", bufs=14))
        h1_p = ctx.enter_context(tc.tile_pool(name="h1", bufs=2))
        o_p = ctx.enter_context(tc.tile_pool(name="osb", bufs=1))
        tp_ps = ctx.enter_context(tc.tile_pool(name="tp_ps", bufs=2, space="PSUM"))
        cv_ps = ctx.enter_context(tc.tile_pool(name="cv_ps", bufs=3, space="PSUM"))
        f1_ps = ctx.enter_context(tc.tile_pool(name="f1_ps", bufs=2, space="PSUM"))
        f2_ps = ctx.enter_context(tc.tile_pool(name="f2_ps", bufs=1, space="PSUM"))

        iden = const.tile([128, 128], bf16, name="iden")
        nc.sync.dma_start(iden[:], id_d)
        kA = const.tile([112, 104], bf16, name="kA")
        nc.sync.dma_start(kA[:], kA_d)
        kB = const.tile([56, 104], bf16, name="kB")
        nc.sync.dma_start(kB[:], kB_d)
        kC = const.tile([112, 52], bf16, name="kC")
        nc.sync.dma_start(kC[:], kC_d)

        def load_f32r(name, shape, src):
            stg = const.tile(shape, f32, tag=f"{name}_stg", name=f"{name}_stg")
            nc.sync.dma_start(stg[:], src)
            t = const.tile(shape, f32r, tag=name, name=name)
            nc.vector.tensor_copy(t[:], stg[:])
            return t

        w1 = []
        offs = 0
        for b in range(7):
            m = 104 if b < 6 else 52
            w1.append(load_f32r(f"w1_{b}", [m, 128], w1_d[offs:offs + m, :]))
            offs += m
        w2 = load_f32r("w2", [128, 10], w2_d)
        b1 = const.tile([128, 1], f32, name="b1")
        nc.sync.dma_start(b1[:], b1_d)
        b2 = const.tile([10, 1], f32, name="b2")
        nc.sync.dma_start(b2[:], b2_d)

        y_sb = o_p.tile([10, b_core], f32, name="y_sb")

        # Phase-separated pairs: all transposes for two batch tiles, then one
        # dense block of 42 real matmuls so the PE clock-gate releases.
        def tp_group(xbm, xfm, t, eng):
            tp = tp_ps.tile([112, BT], bf16, name="tp", tag="tp")
            for c in range(4):
                nc.tensor.transpose(tp[:, c * 128:(c + 1) * 128],
                                    xbm[:, c, 112 * t:112 * t + 112],
                                    iden[:])
            xf = xfm_p.tile([112, BT], bf16, tag="xfm", name=f"xfm{t}")
            if eng == 0:
                nc.vector.tensor_copy(xf[:], tp[:])
            else:
                nc.scalar.copy(xf[:], tp[:])
            xfm.append(xf)

        def conv_block(xfm, hs, b, eng):
            if b < 6:
                cv = cv_ps.tile([104, BT], f32, name="cv", tag="cv")
                nc.tensor.matmul(cv[:], kA[:], xfm[b][:], start=True, stop=False)
                nc.tensor.matmul(cv[:], kB[:], xfm[b + 1][0:56, :],
                                 start=False, stop=True)
                h = h_p.tile([104, BT], f32r, tag="h", name=f"h{b}")
            else:
                cv = cv_ps.tile([52, BT], f32, name="cv6", tag="cv")
                nc.tensor.matmul(cv[:], kC[:], xfm[6][:], start=True, stop=True)
                h = h_p.tile([52, BT], f32r, tag="h", name="h6")
            if eng == 0:
                nc.vector.tensor_scalar_max(h[:], cv[:], 0.0)
            else:
                nc.scalar.activation(h[:], cv[:], AF.Relu)
            hs.append(h)

        for it0 in range(0, nt, 2):
            its = [it0, it0 + 1]
            xfms = []
            hss = []
            for k, it in enumerate(its):
                xbm = xbm_p.tile([128, 4, 784], bf16, name="xbm", tag="xbm")
                src = x_d[it * BT:(it + 1) * BT, :].rearrange(
                    "(c p) d -> p c d", p=128)
                (nc.sync if it % 2 == 0 else nc.scalar).dma_start(xbm[:], src)
                xfm = []
                for t in range(7):
                    tp_group(xbm, xfm, t, (t + k) % 2)
                xfms.append(xfm)
                hss.append([])

            for k in range(2):
                for b in range(7):
                    conv_block(xfms[k], hss[k], b, (b + k) % 2)

            f1s = []
            for k, it in enumerate(its):
                f1s.append(f1_ps.tile([128, BT], f32, name=f"f1_{k}", tag="f1"))
            for b in range(7):
                for k in range(2):
                    nc.tensor.matmul(f1s[k][:], w1[b][:], hss[k][b][:],
                                     start=(b == 0), stop=(b == 6))
            for k, it in enumerate(its):
                h1 = h1_p.tile([128, BT], f32r, name=f"h1_{k}", tag="h1")
                nc.scalar.activation(h1[:], f1s[k][:], AF.Relu, bias=b1[:])
                f2 = f2_ps.tile([10, BT], f32, name=f"f2_{k}", tag="f2")
                nc.tensor.matmul(f2[:], w2[:], h1[:], start=True, stop=True)
                nc.scalar.activation(y_sb[:, it * BT:(it + 1) * BT], f2[:],
                                     AF.Identity, bias=b2[:])

        # ---- single store at the very end (feature-major; host transposes)
        nc.sync.dma_start(y_d, y_sb[:])

    nc.compile()
    return nc


def _host_prep_v4(inputs):
    import ml_dtypes
    bf = ml_dtypes.bfloat16
    x = np.asarray(inputs["x"], dtype=np.float32)
    w = np.asarray(inputs["conv_w"], dtype=np.float32)
    fc1_w = np.asarray(inputs["fc1_w"], dtype=np.float32)
    fc1_b = np.asarray(inputs["fc1_b"], dtype=np.float32)
    fc2_w = np.asarray(inputs["fc2_w"], dtype=np.float32)
    fc2_b = np.asarray(inputs["fc2_b"], dtype=np.float32)

    xp = np.ascontiguousarray(x.astype(bf))

    kA = np.zeros((112, 104), np.float32)
    kB = np.zeros((56, 104), np.float32)
    kC = np.zeros((112, 52), np.float32)
    for oi in range(4):
        for oj in range(26):
            m = oi * 26 + oj
            for di in range(3):
                for dj in range(3):
                    ri, ci = oi + di, oj + dj
                    if ri < 4:
                        kA[ri * 28 + ci, m] = w[di, dj]
                    else:
                        kB[(ri - 4) * 28 + ci, m] = w[di, dj]
    for oi in range(2):
        for oj in range(26):
            m = oi * 26 + oj
            for di in range(3):
                for dj in range(3):
                    kC[(oi + di) * 28 + (oj + dj), m] = w[di, dj]

    consts = {
        "kA": kA.astype(bf),
        "kB": kB.astype(bf),
        "kC": kC.astype(bf),
        "iden": np.eye(128, dtype=np.float32).astype(bf),
        "w1": np.ascontiguousarray(fc1_w.T),
        "b1": np.ascontiguousarray(fc1_b.reshape(128, 1)),
        "w2": np.ascontiguousarray(fc2_w.T),
        "b2": np.ascontiguousarray(fc2_b.reshape(10, 1)),
    }
    in_maps = []
    for c in range(N_CORES):
        m = {"x": xp[c * B_CORE:(c + 1) * B_CORE]}
        m.update(consts)
        in_maps.append(m)
    return in_maps



def _build_module_v6(b_core=B_CORE, n_cores=N_CORES):
    import concourse.bass as bass
    import concourse.tile as tile
    from concourse import bacc, mybir

    f32 = mybir.dt.float32
    f32r = mybir.dt.float32r
    bf16 = mybir.dt.bfloat16
    AF = mybir.ActivationFunctionType
    nt = b_core // BT

    nc = bacc.Bacc("TRN2", target_bir_lowering=False, debug=False,
                   num_devices=n_cores)

    # x arrives feature-major from the host: [7 row-groups, 112 pixels, batch]
    x_d = nc.dram_tensor("x", [7, 112, b_core], bf16, kind="ExternalInput").ap()
    kA_d = nc.dram_tensor("kA", [112, 104], bf16, kind="ExternalInput").ap()
    kB_d = nc.dram_tensor("kB", [56, 104], bf16, kind="ExternalInput").ap()
    kC_d = nc.dram_tensor("kC", [112, 52], bf16, kind="ExternalInput").ap()
    w1_d = nc.dram_tensor("w1", [676, 128], bf16, kind="ExternalInput").ap()
    b1_d = nc.dram_tensor("b1", [128, 1], f32, kind="ExternalInput").ap()
    w2_d = nc.dram_tensor("w2", [128, 10], bf16, kind="ExternalInput").ap()
    b2_d = nc.dram_tensor("b2", [10, 1], f32, kind="ExternalInput").ap()
    y_d = nc.dram_tensor("y", [10, b_core], f32, kind="ExternalOutput").ap()

    with tile.TileContext(nc) as tc, ExitStack() as ctx:
        const = ctx.enter_context(tc.tile_pool(name="const", bufs=1))
        xfm_p = ctx.enter_context(tc.tile_pool(name="xfm", bufs=3))
        h_p = ctx.enter_context(tc.tile_pool(name="h", bufs=14))
        h1_p = ctx.enter_context(tc.tile_pool(name="h1", bufs=2))
        o_p = ctx.enter_context(tc.tile_pool(name="osb", bufs=1))
        cv_ps = ctx.enter_context(tc.tile_pool(name="cv_ps", bufs=4, space="PSUM"))
        f1_ps = ctx.enter_context(tc.tile_pool(name="f1_ps", bufs=2, space="PSUM"))
        f2_ps = ctx.enter_context(tc.tile_pool(name="f2_ps", bufs=2, space="PSUM"))

        kA = const.tile([112, 104], bf16, name="kA")
        nc.sync.dma_start(kA[:], kA_d)
        kB = const.tile([56, 104], bf16, name="kB")
        nc.sync.dma_start(kB[:], kB_d)
        kC = const.tile([112, 52], bf16, name="kC")
        nc.sync.dma_start(kC[:], kC_d)

        w1 = []
        offs = 0
        for b in range(7):
            m = 104 if b < 6 else 52
            t = const.tile([m, 128], bf16, tag=f"w1_{b}", name=f"w1_{b}")
            nc.sync.dma_start(t[:], w1_d[offs:offs + m, :])
            w1.append(t)
            offs += m
        w2 = const.tile([128, 10], bf16, name="w2")
        nc.sync.dma_start(w2[:], w2_d)
        b1 = const.tile([128, 1], f32, name="b1")
        nc.sync.dma_start(b1[:], b1_d)
        b2 = const.tile([10, 1], f32, name="b2")
        nc.sync.dma_start(b2[:], b2_d)

        y_sb = o_p.tile([10, b_core], f32, name="y_sb")

        # Two batch-tiles are processed as interleaved instruction streams:
        # consecutive PE matmuls belong to independent tiles (different PSUM
        # banks, independent deps) so fill/drain phases overlap.
        def emit_pair(its):
            xfms, hss, f1s, h1s = [], [], [], []
            for k, it in enumerate(its):
                xfm = xfm_p.tile([112, 7, BT], bf16, name="xfm", tag="xfm")
                src = x_d[:, :, it * BT:(it + 1) * BT].rearrange("g p b -> p g b")
                (nc.sync if it % 2 == 0 else nc.scalar).dma_start(xfm[:], src)
                xfms.append(xfm)
                hss.append([])

            for b in range(7):
                cvs = []
                if b < 6:
                    for k in range(len(its)):
                        cv = cv_ps.tile([104, BT], f32, name="cv", tag="cv")
                        nc.tensor.matmul(cv[:], kA[:], xfms[k][:, b, :],
                                         start=True, stop=False)
                        cvs.append(cv)
                    for k in range(len(its)):
                        nc.tensor.matmul(cvs[k][:], kB[:],
                                         xfms[k][0:56, b + 1, :],
                                         start=False, stop=True)
                else:
                    for k in range(len(its)):
                        cv = cv_ps.tile([52, BT], f32, name="cv6", tag="cv")
                        nc.tensor.matmul(cv[:], kC[:], xfms[k][:, 6, :],
                                         start=True, stop=True)
                        cvs.append(cv)
                for k in range(len(its)):
                    h = h_p.tile([104 if b < 6 else 52, BT], bf16, tag="h",
                                 name=f"h{b}_{k}")
                    if (b + k) % 2 == 0:
                        nc.vector.tensor_scalar_max(h[:], cvs[k][:], 0.0)
                    else:
                        nc.scalar.activation(h[:], cvs[k][:], AF.Relu)
                    hss[k].append(h)

            for k, it in enumerate(its):
                f1s.append(f1_ps.tile([128, BT], f32, name=f"f1_{k}", tag="f1"))
            for b in range(7):
                for k in range(len(its)):
                    nc.tensor.matmul(f1s[k][:], w1[b][:], hss[k][b][:],
                                     start=(b == 0), stop=(b == 6))
            for k, it in enumerate(its):
                h1 = h1_p.tile([128, BT], bf16, name=f"h1_{k}", tag="h1")
                nc.scalar.activation(h1[:], f1s[k][:], AF.Relu, bias=b1[:])
                h1s.append(h1)
            for k, it in enumerate(its):
                f2 = f2_ps.tile([10, BT], f32, name=f"f2_{k}", tag="f2")
                nc.tensor.matmul(f2[:], w2[:], h1s[k][:], start=True, stop=True)
                nc.scalar.activation(y_sb[:, it * BT:(it + 1) * BT], f2[:],
                                     AF.Identity, bias=b2[:])

        for it2 in range(0, nt, 2):
            emit_pair([it2, it2 + 1])

        # ---- single store at the very end (feature-major; host transposes)
        nc.sync.dma_start(y_d, y_sb[:])

    nc.compile()
    return nc


def _host_prep_v6(inputs):
    import ml_dtypes
    bf = ml_dtypes.bfloat16
    x = np.asarray(inputs["x"], dtype=np.float32)
    w = np.asarray(inputs["conv_w"], dtype=np.float32)
    fc1_w = np.asarray(inputs["fc1_w"], dtype=np.float32)
    fc1_b = np.asarray(inputs["fc1_b"], dtype=np.float32)
    fc2_w = np.asarray(inputs["fc2_w"], dtype=np.float32)
    fc2_b = np.asarray(inputs["fc2_b"], dtype=np.float32)

    B = x.shape[0]
    # feature-major: [7 row-groups, 112 pixels, B]
    xT = np.ascontiguousarray(x.astype(bf).reshape(B, 7, 112).transpose(1, 2, 0))

    kA = np.zeros((112, 104), np.float32)
    kB = np.zeros((56, 104), np.float32)
    kC = np.zeros((112, 52), np.float32)
    for oi in range(4):
        for oj in range(26):
            m = oi * 26 + oj
            for di in range(3):
                for dj in range(3):
                    ri, ci = oi + di, oj + dj
                    if ri < 4:
                        kA[ri * 28 + ci, m] = w[di, dj]
                    else:
                        kB[(ri - 4) * 28 + ci, m] = w[di, dj]
    for oi in range(2):
        for oj in range(26):
            m = oi * 26 + oj
            for di in range(3):
                for dj in range(3):
                    kC[(oi + di) * 28 + (oj + dj), m] = w[di, dj]

    consts = {
        "kA": kA.astype(bf),
        "kB": kB.astype(bf),
        "kC": kC.astype(bf),
        "w1": np.ascontiguousarray(fc1_w.T.astype(bf)),
        "b1": np.ascontiguousarray(fc1_b.reshape(128, 1)),
        "w2": np.ascontiguousarray(fc2_w.T.astype(bf)),
        "b2": np.ascontiguousarray(fc2_b.reshape(10, 1)),
    }
    in_maps = []
    for c in range(N_CORES):
        m = {"x": np.ascontiguousarray(xT[:, :, c * B_CORE:(c + 1) * B_CORE])}
        m.update(consts)
        in_maps.append(m)
    return in_maps


# ---------------------------------------------------------------------------
# v8: misaligned 128-out conv groups (11 conv MMs), dense fc1 (6 MMs), fc2 (1)
# = 18 matmuls per 512-batch tile.  Feature chunks are DMA'd pre-gathered and
# zero-padded so every stationary is [128, 128] (FWL-eligible) and every
# moving operand is a contiguous [128, 512] slice.  Software-pipelined skew:
# PE stream per tile = [fc2(t-2)] [conv(t) x11] [fc1(t-1) x6].
# ---------------------------------------------------------------------------

# chunk -> (pixel start, valid size); 2 chunks per 128-out group, 1 for the
# final 36-out group.  Receptive rows per group: g covers out rows
# [o//26 .. (o+m-1)//26 + 2] -> pixel span split into <=128-partition chunks.
V8_CHUNKS = [(0, 128), (128, 68), (112, 128), (240, 96), (252, 128), (380, 96),
             (392, 128), (520, 96), (532, 128), (660, 96), (672, 112)]
V8_CHUNK_GROUP = [0, 0, 1, 1, 2, 2, 3, 3, 4, 4, 5]
V8_GROUP_OUT = [(0, 128), (128, 128), (256, 128), (384, 128), (512, 128),
                (640, 36)]
V8_NCH = 11
V8_WORDS = V8_NCH * BT  # 5632 moving words per tile per partition


def _build_module_v8(b_core=B_CORE, n_cores=N_CORES, prefetch=3):
    import concourse.bass as bass
    import concourse.tile as tile
    from concourse import bacc, mybir

    f32 = mybir.dt.float32
    bf16 = mybir.dt.bfloat16
    AF = mybir.ActivationFunctionType
    nt = b_core // BT

    nc = bacc.Bacc("TRN2", target_bir_lowering=False, debug=False,
                   num_devices=n_cores)

    x_d = nc.dram_tensor("x", [nt, 128, V8_WORDS], bf16,
                         kind="ExternalInput").ap()
    # all bf16 weights in one blob: 11 conv chunks + 6 fc1 chunks ([128,128]
    # each) + fc2 [128,10] concatenated along free dim
    wb_d = nc.dram_tensor("wblob", [128, 17 * 128 + 10], bf16,
                          kind="ExternalInput").ap()
    bias_d = nc.dram_tensor("biases", [128, 2], f32, kind="ExternalInput").ap()
    y_d = nc.dram_tensor("y", [10, b_core], f32, kind="ExternalOutput").ap()

    with tile.TileContext(nc) as tc, ExitStack() as ctx:
        const = ctx.enter_context(tc.tile_pool(name="const", bufs=1))
        xg_p = ctx.enter_context(tc.tile_pool(name="xg", bufs=prefetch + 1))
        h_p = ctx.enter_context(tc.tile_pool(name="h", bufs=12))
        h1_p = ctx.enter_context(tc.tile_pool(name="h1", bufs=2))
        o_p = ctx.enter_context(tc.tile_pool(name="osb", bufs=1))
        cv_ps = ctx.enter_context(tc.tile_pool(name="cv_ps", bufs=5, space="PSUM"))
        f1_ps = ctx.enter_context(tc.tile_pool(name="f1_ps", bufs=2, space="PSUM"))
        f2_ps = ctx.enter_context(tc.tile_pool(name="f2_ps", bufs=1, space="PSUM"))

        wblob = const.tile([128, 17 * 128 + 10], bf16, name="wblob")
        nc.sync.dma_start(wblob[:], wb_d)
        biases = const.tile([128, 2], f32, name="biases")
        nc.sync.dma_start(biases[:], bias_d)
        wc = [wblob[:, c * 128:(c + 1) * 128] for c in range(V8_NCH)]
        w1 = [wblob[:, (V8_NCH + c) * 128:(V8_NCH + c + 1) * 128]
              for c in range(6)]
        w2 = wblob[:, 17 * 128:17 * 128 + 10]
        b1 = biases[:, 0:1]
        b2 = biases[0:10, 1:2]

        y_sb = o_p.tile([10, b_core], f32, name="y_sb")

        xgs = []

        def dma_x(t):
            xg = xg_p.tile([128, V8_WORDS], bf16, tag="xg", name=f"xg{t}")
            nc.sync.dma_start(xg[:], x_d[t])
            xgs.append(xg)

        hss = {}   # t -> list of 6 h tiles
        h1s = {}   # t -> h1 tile

        def emit_conv(t):
            xg = xgs.pop(0)
            hs = []
            for g in range(6):
                cv = cv_ps.tile([128, BT], f32, tag="cv", name=f"cv{g}")
                c0 = 2 * g
                if g < 5:
                    nc.tensor.matmul(cv[:], wc[c0], xg[:, c0 * BT:(c0 + 1) * BT],
                                     start=True, stop=False)
                    nc.tensor.matmul(cv[:], wc[c0 + 1],
                                     xg[:, (c0 + 1) * BT:(c0 + 2) * BT],
                                     start=False, stop=True)
                else:
                    nc.tensor.matmul(cv[:], wc[10], xg[:, 10 * BT:11 * BT],
                                     start=True, stop=True)
                h = h_p.tile([128, BT], bf16, tag="h", name=f"h{g}")
                if g % 2 == 0:
                    nc.vector.tensor_scalar_max(h[:], cv[:], 0.0)
                else:
                    nc.scalar.activation(h[:], cv[:], AF.Relu)
                hs.append(h)
            hss[t] = hs

        def emit_fc1(t):
            hs = hss.pop(t)
            f1 = f1_ps.tile([128, BT], f32, tag="f1", name="f1")
            for c in range(6):
                nc.tensor.matmul(f1[:], w1[c], hs[c][:],
                                 start=(c == 0), stop=(c == 5))
            h1 = h1_p.tile([128, BT], bf16, tag="h1", name="h1")
            nc.scalar.activation(h1[:], f1[:], AF.Relu, bias=b1)
            h1s[t] = h1

        def emit_fc2(t):
            h1 = h1s.pop(t)
            f2 = f2_ps.tile([10, BT], f32, tag="f2", name="f2")
            nc.tensor.matmul(f2[:], w2, h1[:], start=True, stop=True)
            nc.vector.tensor_scalar_add(y_sb[:, t * BT:(t + 1) * BT], f2[:], b2)

        for t in range(prefetch):
            dma_x(t)
        for t in range(nt):
            if t + prefetch < nt:
                dma_x(t + prefetch)
            if t >= 2:
                emit_fc2(t - 2)
            emit_conv(t)
            if t >= 1:
                emit_fc1(t - 1)
            # stream out finished quarter of y while compute continues
            if t % 4 == 3 and t >= 7:
                q = t // 4 - 1  # quarter fully written: tiles 4q..4q+3
                nc.sync.dma_start(y_d[:, q * 4 * BT:(q + 1) * 4 * BT],
                                  y_sb[:, q * 4 * BT:(q + 1) * 4 * BT])
        emit_fc1(nt - 1)
        emit_fc2(nt - 2)
        emit_fc2(nt - 1)
        nc.sync.dma_start(y_d[:, 12 * BT:], y_sb[:, 12 * BT:])

    nc.compile()
    return nc


def _host_prep_v8(inputs):
    import ml_dtypes
    bf = ml_dtypes.bfloat16
    x = np.asarray(inputs["x"], dtype=np.float32)
    w = np.asarray(inputs["conv_w"], dtype=np.float32)
    fc1_w = np.asarray(inputs["fc1_w"], dtype=np.float32)
    fc1_b = np.asarray(inputs["fc1_b"], dtype=np.float32)
    fc2_w = np.asarray(inputs["fc2_w"], dtype=np.float32)
    fc2_b = np.asarray(inputs["fc2_b"], dtype=np.float32)
    B = x.shape[0]
    nt = B_CORE // BT

    # gather pixels into zero-padded 128-partition chunks
    idx = np.full(V8_NCH * 128, 784, np.int64)
    for c, (s, sz) in enumerate(V8_CHUNKS):
        idx[c * 128:c * 128 + sz] = s + np.arange(sz)
    xb = np.concatenate([x.astype(bf), np.zeros((B, 1), bf)], axis=1)[:, idx]
    # [B, 11*128] -> [core, tile, partition, chunk, sample]
    xb = xb.reshape(N_CORES, nt, BT, V8_NCH, 128).transpose(0, 1, 4, 3, 2)
    xb = np.ascontiguousarray(xb).reshape(N_CORES, nt, 128, V8_WORDS)

    # conv band stationaries [11, 128, 128]
    wcs = np.zeros((V8_NCH, 128, 128), np.float32)
    p = np.arange(128)
    j = np.arange(128)
    for c, (start, size) in enumerate(V8_CHUNKS):
        g = V8_CHUNK_GROUP[c]
        ostart, osz = V8_GROUP_OUT[g]
        px = start + p
        o = ostart + j
        di = (px // 28)[:, None] - (o // 26)[None, :]
        dj = (px % 28)[:, None] - (o % 26)[None, :]
        m = ((di >= 0) & (di < 3) & (dj >= 0) & (dj < 3)
             & (p < size)[:, None] & (j < osz)[None, :])
        wcs[c][m] = w[np.clip(di, 0, 2), np.clip(dj, 0, 2)][m]

    # fc1 chunks [6, 128, 128] (rows grouped to match h layout, zero-padded)
    w1g = np.zeros((6, 128, 128), np.float32)
    for g, (os_, sz) in enumerate(V8_GROUP_OUT):
        w1g[g, :sz, :] = fc1_w.T[os_:os_ + sz, :]

    wblob = np.concatenate(
        [wcs.transpose(1, 0, 2).reshape(128, V8_NCH * 128),
         w1g.transpose(1, 0, 2).reshape(128, 6 * 128),
         fc2_w.T], axis=1).astype(bf)
    biases = np.zeros((128, 2), np.float32)
    biases[:, 0] = fc1_b
    biases[:10, 1] = fc2_b

    in_maps = []
    for c in range(N_CORES):
        in_maps.append({"x": xb[c], "wblob": wblob, "biases": biases})
    return in_maps


# ---------------------------------------------------------------------------
# v9: like v8 but x is stored as 8 overlapping 128-pixel columns (no zero
# padding, 1024 words/sample vs 1408) chosen so every conv group's receptive
# span is covered by 2 columns (1 for the last).  Stationaries zero out taps
# outside each MM's assigned pixel set (disjoint across a group's two MMs).
# Consts go on the scalar DMA queue (overlaps the first x DMA on sync), and
# 8 warmup matmuls on the weight blob flip the PE HAM clock gate to 8/8
# while the first x tile is still in flight.
# ---------------------------------------------------------------------------

V9_COLS = [0, 112, 220, 348, 392, 500, 628, 656]
V9_GROUP_COLS = [(0, 1), (1, 2), (2, 3), (4, 5), (5, 6), (7,)]
V9_NCOL = 8
V9_WORDS = V9_NCOL * BT  # 4096


def _build_module_v9(b_core=B_CORE, n_cores=N_CORES, prefetch=3, warmup=10):
    import concourse.bass as bass
    import concourse.tile as tile
    from concourse import bacc, mybir

    f32 = mybir.dt.float32
    bf16 = mybir.dt.bfloat16
    AF = mybir.ActivationFunctionType
    nt = b_core // BT
    nmm = 11  # conv matmuls per tile

    nc = bacc.Bacc("TRN2", target_bir_lowering=False, debug=False,
                   num_devices=n_cores)

    x_d = nc.dram_tensor("x", [nt, 128, V9_WORDS], bf16,
                         kind="ExternalInput").ap()
    wb_d = nc.dram_tensor("wblob", [128, (nmm + 6) * 128 + 10], bf16,
                          kind="ExternalInput").ap()
    bias_d = nc.dram_tensor("biases", [128, 2], f32, kind="ExternalInput").ap()
    y_d = nc.dram_tensor("y", [10, b_core], f32, kind="ExternalOutput").ap()

    with tile.TileContext(nc) as tc, ExitStack() as ctx:
        const = ctx.enter_context(tc.tile_pool(name="const", bufs=1))
        xg_p = ctx.enter_context(tc.tile_pool(name="xg", bufs=prefetch + 1))
        h_p = ctx.enter_context(tc.tile_pool(name="h", bufs=12))
        h1_p = ctx.enter_context(tc.tile_pool(name="h1", bufs=2))
        o_p = ctx.enter_context(tc.tile_pool(name="osb", bufs=1))
        cv_ps = ctx.enter_context(tc.tile_pool(name="cv_ps", bufs=6, space="PSUM"))
        f1_ps = ctx.enter_context(tc.tile_pool(name="f1_ps", bufs=1, space="PSUM"))
        f2_ps = ctx.enter_context(tc.tile_pool(name="f2_ps", bufs=1, space="PSUM"))

        # warm up the PE HAM clock gate on an on-chip zero tile (no DMA dep)
        warm_src = const.tile([128, BT], bf16, name="warm_src")
        nc.gpsimd.memset(warm_src[:], 0)
        for _ in range(warmup):
            wm = f1_ps.tile([128, BT], f32, tag="f1", name="warm")
            nc.tensor.matmul(wm[:], warm_src[:, 0:128], warm_src[:],
                             start=True, stop=True)

        wblob = const.tile([128, (nmm + 6) * 128 + 10], bf16, name="wblob")
        # conv weights first so the first conv matmul's dep is small
        nc.scalar.dma_start(wblob[:, :nmm * 128], wb_d[:, :nmm * 128])
        nc.scalar.dma_start(wblob[:, nmm * 128:], wb_d[:, nmm * 128:])
        biases = const.tile([128, 2], f32, name="biases")
        nc.scalar.dma_start(biases[:], bias_d)
        wc = [wblob[:, c * 128:(c + 1) * 128] for c in range(nmm)]
        w1 = [wblob[:, (nmm + c) * 128:(nmm + c + 1) * 128] for c in range(6)]
        w2 = wblob[:, (nmm + 6) * 128:(nmm + 6) * 128 + 10]
        b1 = biases[:, 0:1]
        b2 = biases[0:10, 1:2]

        y_sb = o_p.tile([10, b_core], f32, name="y_sb")

        xgs = []

        def dma_x(t, split=False):
            xg = xg_p.tile([128, V9_WORDS], bf16, tag="xg", name=f"xg{t}")
            if split:
                # conv g0 needs cols 0-1, g1 needs 1-2; rest arrive second
                for lo, hi in ((0, 3), (3, 8)):
                    nc.sync.dma_start(xg[:, lo * BT:hi * BT],
                                      x_d[t][:, lo * BT:hi * BT])
            else:
                nc.sync.dma_start(xg[:], x_d[t])
            xgs.append(xg)

        hss = {}
        h1s = {}

        def emit_conv(t):
            xg = xgs.pop(0)
            mi = 0
            hs = []
            for g in range(6):
                cols = V9_GROUP_COLS[g]
                cv = cv_ps.tile([128, BT], f32, tag="cv", name=f"cv{g}")
                for k, col in enumerate(cols):
                    nc.tensor.matmul(cv[:], wc[mi],
                                     xg[:, col * BT:(col + 1) * BT],
                                     start=(k == 0), stop=(k == len(cols) - 1))
                    mi += 1
                h = h_p.tile([128, BT], bf16, tag="h", name=f"h{g}")
                if g % 2 == 0:
                    nc.vector.tensor_scalar_max(h[:], cv[:], 0.0)
                else:
                    nc.scalar.activation(h[:], cv[:], AF.Relu)
                hs.append(h)
            hss[t] = hs

        def emit_fc1(t):
            hs = hss.pop(t)
            f1 = f1_ps.tile([128, BT], f32, tag="f1", name="f1")
            for c in range(6):
                nc.tensor.matmul(f1[:], w1[c], hs[c][:],
                                 start=(c == 0), stop=(c == 5))
            h1 = h1_p.tile([128, BT], bf16, tag="h1", name="h1")
            nc.scalar.activation(h1[:], f1[:], AF.Relu, bias=b1)
            h1s[t] = h1

        def emit_fc2(t):
            h1 = h1s.pop(t)
            f2 = f2_ps.tile([10, BT], f32, tag="f2", name="f2")
            nc.tensor.matmul(f2[:], w2, h1[:], start=True, stop=True)
            nc.vector.tensor_scalar_add(y_sb[:, t * BT:(t + 1) * BT], f2[:], b2)

        for t in range(prefetch):
            dma_x(t, split=(t == 0))
        for t in range(nt):
            if t + prefetch < nt:
                dma_x(t + prefetch)
            if t >= 2:
                emit_fc2(t - 2)
            emit_conv(t)
            if t >= 1:
                emit_fc1(t - 1)
            if t % 4 == 3 and t >= 7:
                q = t // 4 - 1
                nc.sync.dma_start(y_d[:, q * 4 * BT:(q + 1) * 4 * BT],
                                  y_sb[:, q * 4 * BT:(q + 1) * 4 * BT])
        emit_fc1(nt - 1)
        emit_fc2(nt - 2)
        nc.sync.dma_start(y_d[:, 12 * BT:15 * BT], y_sb[:, 12 * BT:15 * BT])
        emit_fc2(nt - 1)
        nc.sync.dma_start(y_d[:, 15 * BT:], y_sb[:, 15 * BT:])

    nc.compile()
    return nc


def _host_prep_v9(inputs):
    import ml_dtypes
    bf = ml_dtypes.bfloat16
    x = np.asarray(inputs["x"], dtype=np.float32)
    w = np.asarray(inputs["conv_w"], dtype=np.float32)
    fc1_w = np.asarray(inputs["fc1_w"], dtype=np.float32)
    fc1_b = np.asarray(inputs["fc1_b"], dtype=np.float32)
    fc2_w = np.asarray(inputs["fc2_w"], dtype=np.float32)
    fc2_b = np.asarray(inputs["fc2_b"], dtype=np.float32)
    B = x.shape[0]
    nt = B_CORE // BT

    idx = np.concatenate([np.arange(a, a + 128) for a in V9_COLS])
    xb = x.astype(bf)[:, idx]
    xb = xb.reshape(N_CORES, nt, BT, V9_NCOL, 128).transpose(0, 1, 4, 3, 2)
    xb = np.ascontiguousarray(xb).reshape(N_CORES, nt, 128, V9_WORDS)

    # conv stationaries: per group, taps assigned to the first column that
    # contains the pixel (disjoint coverage across the group's matmuls)
    p = np.arange(128)
    j = np.arange(128)
    wcs = []
    for g, cols in enumerate(V9_GROUP_COLS):
        ostart, osz = V8_GROUP_OUT[g]
        covered_lo = None  # pixel range already handled by earlier col
        for col in cols:
            a = V9_COLS[col]
            px = a + p
            o = ostart + j
            di = (px // 28)[:, None] - (o // 26)[None, :]
            dj = (px % 28)[:, None] - (o % 26)[None, :]
            m = ((di >= 0) & (di < 3) & (dj >= 0) & (dj < 3)
                 & (j < osz)[None, :])
            if covered_lo is not None:
                lo, hi = covered_lo
                m &= ~((px >= lo) & (px < hi))[:, None]
            W = np.zeros((128, 128), np.float32)
            W[m] = w[np.clip(di, 0, 2), np.clip(dj, 0, 2)][m]
            wcs.append(W)
            covered_lo = (a, a + 128) if covered_lo is None else \
                (min(covered_lo[0], a), max(covered_lo[1], a + 128))
    wcs = np.stack(wcs)  # [11, 128, 128]

    w1g = np.zeros((6, 128, 128), np.float32)
    for g, (os_, sz) in enumerate(V8_GROUP_OUT):
        w1g[g, :sz, :] = fc1_w.T[os_:os_ + sz, :]

    wblob = np.concatenate(
        [wcs.transpose(1, 0, 2).reshape(128, 11 * 128),
         w1g.transpose(1, 0, 2).reshape(128, 6 * 128),
         fc2_w.T], axis=1).astype(bf)
    biases = np.zeros((128, 2), np.float32)
    biases[:, 0] = fc1_b
    biases[:10, 1] = fc2_b

    in_maps = []
    for c in range(N_CORES):
        in_maps.append({"x": xb[c], "wblob": wblob, "biases": biases})
    return in_maps


VERSION = 9


def run(inputs, trace=False, tmpdir=None, version=None):
    from concourse.bass_utils import run_bass_kernel_spmd

    version = VERSION if version is None else version
    key = f"nc{version}"
    builders = {9: _build_module_v9, 8: _build_module_v8, 6: _build_module_v6,
                4: _build_module_v4, 2: _build_module}
    preps = {9: _host_prep_v9, 8: _host_prep_v8, 6: _host_prep_v6,
             4: _host_prep_v4, 2: _host_prep}
    if key not in _cache:
        _cache[key] = builders[version]()
    nc = _cache[key]
    in_maps = preps[version](inputs)
    res = run_bass_kernel_spmd(nc, in_maps, list(range(N_CORES)), trace=trace,
                               tmpdir=tmpdir)
    out = np.concatenate([np.ascontiguousarray(r["y"].T) for r in res.results], axis=0)
    return out, res


def kernel(**inputs) -> np.ndarray:
    out, _ = run(inputs, trace=False)
    return out



# revision 21
# speedup vs baseline: 1.0283x; 1.0283x over previous
"""Trainium2 Bass kernel for DigitConvolutionalModel.

Pipeline (per core, pure data-parallel over batch):
  x [8192, 784] --DMA--> SBUF batch-major --PE transpose--> feature-major tiles
  conv 3x3 as banded block-matmuls on PE -> relu -> fc1 (matmul) -> relu
  -> fc2 (matmul) + bias -> DMA out.

All activations live feature-major ([features, batch]) so the PE can contract
over the partition dim. The 3x3 conv is expressed as 13 small banded matmuls
per 512-batch tile using three constant band matrices built on the host from
conv_w (shift-invariant across 4-image-row blocks).
"""

import numpy as np
from contextlib import ExitStack

N_CORES = 8
B_FULL = 65536
B_CORE = B_FULL // N_CORES  # 8192
BT = 512                    # batch tile (matmul moving free dim)
NT = B_CORE // BT           # 16

_cache = {}


def _build_module(b_core=B_CORE, n_cores=N_CORES):
    import concourse.bass as bass
    import concourse.tile as tile
    from concourse import bacc, mybir

    f32 = mybir.dt.float32
    f32r = mybir.dt.float32r
    AF = mybir.ActivationFunctionType
    nt = b_core // BT

    nc = bacc.Bacc("TRN2", target_bir_lowering=False, debug=False,
                   num_devices=n_cores)

    x_d = nc.dram_tensor("x", [b_core, 784], f32, kind="ExternalInput").ap()
    kA_d = nc.dram_tensor("kA", [112, 104], f32, kind="ExternalInput").ap()
    kB_d = nc.dram_tensor("kB", [56, 104], f32, kind="ExternalInput").ap()
    kC_d = nc.dram_tensor("kC", [112, 52], f32, kind="ExternalInput").ap()
    w1_d = nc.dram_tensor("w1", [676, 128], f32, kind="ExternalInput").ap()
    b1_d = nc.dram_tensor("b1", [128, 1], f32, kind="ExternalInput").ap()
    w2_d = nc.dram_tensor("w2", [128, 10], f32, kind="ExternalInput").ap()
    b2_d = nc.dram_tensor("b2", [10, 1], f32, kind="ExternalInput").ap()
    id_d = nc.dram_tensor("iden", [128, 128], f32, kind="ExternalInput").ap()
    y_d = nc.dram_tensor("y", [10, b_core], f32, kind="ExternalOutput").ap()

    with tile.TileContext(nc) as tc, ExitStack() as ctx:
        const = ctx.enter_context(tc.tile_pool(name="const", bufs=1))
        xbm_p = ctx.enter_context(tc.tile_pool(name="xbm", bufs=4))
        xfm_p = ctx.enter_context(tc.tile_pool(name="xfm", bufs=21))
        h_p = ctx.enter_context(tc.tile_pool(name="h", bufs=14))
        h1_p = ctx.enter_context(tc.tile_pool(name="h1", bufs=2))
        o_p = ctx.enter_context(tc.tile_pool(name="osb", bufs=2))
        tp_ps = ctx.enter_context(tc.tile_pool(name="tp_ps", bufs=2, space="PSUM"))
        cv_ps = ctx.enter_context(tc.tile_pool(name="cv_ps", bufs=2, space="PSUM"))
        f1_ps = ctx.enter_context(tc.tile_pool(name="f1_ps", bufs=2, space="PSUM"))
        f2_ps = ctx.enter_context(tc.tile_pool(name="f2_ps", bufs=2, space="PSUM"))

        iden = const.tile([128, 128], f32, name="iden")
        nc.sync.dma_start(iden[:], id_d)

        def load_f32r(name, shape, src):
            stg = const.tile(shape, f32, tag=f"{name}_stg", name=f"{name}_stg")
            nc.sync.dma_start(stg[:], src)
            t = const.tile(shape, f32r, tag=name, name=name)
            nc.vector.tensor_copy(t[:], stg[:])
            return t

        kA = load_f32r("kA", [112, 104], kA_d)
        kB = load_f32r("kB", [56, 104], kB_d)
        kC = load_f32r("kC", [112, 52], kC_d)
        w1 = []
        offs = 0
        for b in range(7):
            m = 104 if b < 6 else 52
            w1.append(load_f32r(f"w1_{b}", [m, 128], w1_d[offs:offs + m, :]))
            offs += m
        w2 = load_f32r("w2", [128, 10], w2_d)
        b1 = const.tile([128, 1], f32, name="b1")
        nc.sync.dma_start(b1[:], b1_d)
        b2 = const.tile([10, 1], f32, name="b2")
        nc.sync.dma_start(b2[:], b2_d)

        for it in range(nt):
            # ---- load one batch tile, batch-major [128, 4, 784]
            xbm = xbm_p.tile([128, 4, 784], f32, name="xbm", tag="xbm")
            src = x_d[it * BT:(it + 1) * BT, :].rearrange("(c p) d -> p c d", p=128)
            (nc.sync if it % 2 == 0 else nc.scalar).dma_start(xbm[:], src)

            # ---- transpose to feature-major tiles xfm[t] = x.T rows 112t..112t+111
            xfm = []
            for t in range(7):
                tp = tp_ps.tile([112, BT], f32, name="tp", tag="tp")
                for c in range(4):
                    nc.tensor.transpose(tp[:, c * 128:(c + 1) * 128],
                                        xbm[:, c, 112 * t:112 * t + 112],
                                        iden[:])
                xf = xfm_p.tile([112, BT], f32r, tag="xfm", name=f"xfm{t}")
                if t % 2 == 0:
                    nc.vector.tensor_copy(xf[:], tp[:])
                else:
                    nc.scalar.copy(xf[:], tp[:])
                xfm.append(xf)

            # ---- conv as banded matmuls, relu into h blocks
            hs = []
            for b in range(6):
                cv = cv_ps.tile([104, BT], f32, name="cv", tag="cv")
                nc.tensor.matmul(cv[:], kA[:], xfm[b][:], start=True, stop=False)
                nc.tensor.matmul(cv[:], kB[:], xfm[b + 1][0:56, :],
                                 start=False, stop=True)
                h = h_p.tile([104, BT], f32r, tag="h", name=f"h{b}")
                if b % 2 == 0:
                    nc.vector.tensor_scalar_max(h[:], cv[:], 0.0)
                else:
                    nc.scalar.activation(h[:], cv[:], AF.Relu)
                hs.append(h)
            cv = cv_ps.tile([52, BT], f32, name="cv6", tag="cv")
            nc.tensor.matmul(cv[:], kC[:], xfm[6][:], start=True, stop=True)
            h = h_p.tile([52, BT], f32r, tag="h", name="h6")
            nc.vector.tensor_scalar_max(h[:], cv[:], 0.0)
            hs.append(h)

            # ---- fc1: accumulate 7 chunks, relu + bias
            f1 = f1_ps.tile([128, BT], f32, name="f1", tag="f1")
            for b in range(7):
                nc.tensor.matmul(f1[:], w1[b][:], hs[b][:],
                                 start=(b == 0), stop=(b == 6))
            h1 = h1_p.tile([128, BT], f32r, name="h1", tag="h1")
            nc.scalar.activation(h1[:], f1[:], AF.Relu, bias=b1[:])

            # ---- fc2 + bias
            f2 = f2_ps.tile([10, BT], f32, name="f2", tag="f2")
            nc.tensor.matmul(f2[:], w2[:], h1[:], start=True, stop=True)
            osb = o_p.tile([10, BT], f32, name="osb", tag="osb")
            nc.scalar.activation(osb[:], f2[:], AF.Identity, bias=b2[:])

            # ---- store (feature-major; host transposes)
            nc.sync.dma_start(y_d[:, it * BT:(it + 1) * BT], osb[:])

    nc.compile()
    return nc


def _host_prep(inputs):
    x = np.ascontiguousarray(np.asarray(inputs["x"], dtype=np.float32))
    w = np.asarray(inputs["conv_w"], dtype=np.float32)
    fc1_w = np.asarray(inputs["fc1_w"], dtype=np.float32)
    fc1_b = np.asarray(inputs["fc1_b"], dtype=np.float32)
    fc2_w = np.asarray(inputs["fc2_w"], dtype=np.float32)
    fc2_b = np.asarray(inputs["fc2_b"], dtype=np.float32)

    kA = np.zeros((112, 104), np.float32)
    kB = np.zeros((56, 104), np.float32)
    kC = np.zeros((112, 52), np.float32)
    for oi in range(4):
        for oj in range(26):
            m = oi * 26 + oj
            for di in range(3):
                for dj in range(3):
                    ri, ci = oi + di, oj + dj
                    if ri < 4:
                        kA[ri * 28 + ci, m] = w[di, dj]
                    else:
                        kB[(ri - 4) * 28 + ci, m] = w[di, dj]
    for oi in range(2):
        for oj in range(26):
            m = oi * 26 + oj
            for di in range(3):
                for dj in range(3):
                    kC[(oi + di) * 28 + (oj + dj), m] = w[di, dj]

    consts = {
        "kA": kA,
        "kB": kB,
        "kC": kC,
        "w1": np.ascontiguousarray(fc1_w.T),
        "b1": np.ascontiguousarray(fc1_b.reshape(128, 1)),
        "w2": np.ascontiguousarray(fc2_w.T),
        "b2": np.ascontiguousarray(fc2_b.reshape(10, 1)),
        "iden": np.eye(128, dtype=np.float32),
    }
    in_maps = []
    for c in range(N_CORES):
        m = {"x": x[c * B_CORE:(c + 1) * B_CORE]}
        m.update(consts)
        in_maps.append(m)
    return in_maps


GBT = 2048                  # batch rows per DMA-transpose group (4 tiles)


def _build_module_v4(b_core=B_CORE, n_cores=N_CORES):
    import concourse.bass as bass
    import concourse.tile as tile
    from concourse import bacc, mybir

    f32 = mybir.dt.float32
    f32r = mybir.dt.float32r
    bf16 = mybir.dt.bfloat16
    AF = mybir.ActivationFunctionType
    nt = b_core // BT

    nc = bacc.Bacc("TRN2", target_bir_lowering=False, debug=False,
                   num_devices=n_cores)

    x_d = nc.dram_tensor("x", [b_core, 784], bf16, kind="ExternalInput").ap()
    kA_d = nc.dram_tensor("kA", [112, 104], bf16, kind="ExternalInput").ap()
    kB_d = nc.dram_tensor("kB", [56, 104], bf16, kind="ExternalInput").ap()
    kC_d = nc.dram_tensor("kC", [112, 52], bf16, kind="ExternalInput").ap()
    id_d = nc.dram_tensor("iden", [128, 128], bf16, kind="ExternalInput").ap()
    w1_d = nc.dram_tensor("w1", [676, 128], f32, kind="ExternalInput").ap()
    b1_d = nc.dram_tensor("b1", [128, 1], f32, kind="ExternalInput").ap()
    w2_d = nc.dram_tensor("w2", [128, 10], f32, kind="ExternalInput").ap()
    b2_d = nc.dram_tensor("b2", [10, 1], f32, kind="ExternalInput").ap()
    y_d = nc.dram_tensor("y", [10, b_core], f32, kind="ExternalOutput").ap()

    with tile.TileContext(nc) as tc, ExitStack() as ctx:
        const = ctx.enter_context(tc.tile_pool(name="const", bufs=1))
        xbm_p = ctx.enter_context(tc.tile_pool(name="xbm", bufs=4))
        xfm_p = ctx.enter_context(tc.tile_pool(name="xfm", bufs=21))
        h_p = ctx.enter_context(tc.tile_pool(name="h", bufs=14))
        h1_p = ctx.enter_context(tc.tile_pool(name="h1", bufs=2))
        o_p = ctx.enter_context(tc.tile_pool(name="osb", bufs=1))
        tp_ps = ctx.enter_context(tc.tile_pool(name="tp_ps", bufs=2, space="PSUM"))
        cv_ps = ctx.enter_context(tc.tile_pool(name="cv_ps", bufs=3, space="PSUM"))
        f1_ps = ctx.enter_context(tc.tile_pool(name="f1_ps", bufs=2, space="PSUM"))
        f2_ps = ctx.enter_context(tc.tile_pool(name="f2_ps", bufs=1, space="PSUM"))

        iden = const.tile([128, 128], bf16, name="iden")
        nc.sync.dma_start(iden[:], id_d)
        kA = const.tile([112, 104], bf16, name="kA")
        nc.sync.dma_start(kA[:], kA_d)
        kB = const.tile([56, 104], bf16, name="kB")
        nc.sync.dma_start(kB[:], kB_d)
        kC = const.tile([112, 52], bf16, name="kC")
        nc.sync.dma_start(kC[:], kC_d)

        def load_f32r(name, shape, src):
            stg = const.tile(shape, f32, tag=f"{name}_stg", name=f"{name}_stg")
            nc.sync.dma_start(stg[:], src)
            t = const.tile(shape, f32r, tag=name, name=name)
            nc.vector.tensor_copy(t[:], stg[:])
            return t

        w1 = []
        offs = 0
        for b in range(7):
            m = 104 if b < 6 else 52
            w1.append(load_f32r(f"w1_{b}", [m, 128], w1_d[offs:offs + m, :]))
            offs += m
        w2 = load_f32r("w2", [128, 10], w2_d)
        b1 = const.tile([128, 1], f32, name="b1")
        nc.sync.dma_start(b1[:], b1_d)
        b2 = const.tile([10, 1], f32, name="b2")
        nc.sync.dma_start(b2[:], b2_d)

        y_sb = o_p.tile([10, b_core], f32, name="y_sb")

        # Phase-separated pairs: all transposes for two batch tiles, then one
        # dense block of 42 real matmuls so the PE clock-gate releases.
        def tp_group(xbm, xfm, t, eng):
            tp = tp_ps.tile([112, BT], bf16, name="tp", tag="tp")
            for c in range(4):
                nc.tensor.transpose(tp[:, c * 128:(c + 1) * 128],
                                    xbm[:, c, 112 * t:112 * t + 112],
                                    iden[:])
            xf = xfm_p.tile([112, BT], bf16, tag="xfm", name=f"xfm{t}")
            if eng == 0:
                nc.vector.tensor_copy(xf[:], tp[:])
            else:
                nc.scalar.copy(xf[:], tp[:])
            xfm.append(xf)

        def conv_block(xfm, hs, b, eng):
            if b < 6:
                cv = cv_ps.tile([104, BT], f32, name="cv", tag="cv")
                nc.tensor.matmul(cv[:], kA[:], xfm[b][:], start=True, stop=False)
                nc.tensor.matmul(cv[:], kB[:], xfm[b + 1][0:56, :],
                                 start=False, stop=True)
                h = h_p.tile([104, BT], f32r, tag="h", name=f"h{b}")
            else:
                cv = cv_ps.tile([52, BT], f32, name="cv6", tag="cv")
                nc.tensor.matmul(cv[:], kC[:], xfm[6][:], start=True, stop=True)
                h = h_p.tile([52, BT], f32r, tag="h", name="h6")
            if eng == 0:
                nc.vector.tensor_scalar_max(h[:], cv[:], 0.0)
            else:
                nc.scalar.activation(h[:], cv[:], AF.Relu)
            hs.append(h)

        for it0 in range(0, nt, 2):
            its = [it0, it0 + 1]
            xfms = []
            hss = []
            for k, it in enumerate(its):
                xbm = xbm_p.tile([128, 4, 784], bf16, name="xbm", tag="xbm")
                src = x_d[it * BT:(it + 1) * BT, :].rearrange(
                    "(c p) d -> p c d", p=128)
                (nc.sync if it % 2 == 0 else nc.scalar).dma_start(xbm[:], src)
                xfm = []
                for t in range(7):
                    tp_group(xbm, xfm, t, (t + k) % 2)
                xfms.append(xfm)
                hss.append([])

            for k in range(2):
                for b in range(7):
                    conv_block(xfms[k], hss[k], b, (b + k) % 2)

            f1s = []
            for k, it in enumerate(its):
                f1s.append(f1_ps.tile([128, BT], f32, name=f"f1_{k}", tag="f1"))
            for b in range(7):
                for k in range(2):
                    nc.tensor.matmul(f1s[k][:], w1[b][:], hss[k][b][:],
                                     start=(b == 0), stop=(b == 6))
            for k, it in enumerate(its):
                h1 = h1_p.tile([128, BT], f32r, name=f"h1_{k}", tag="h1")
                nc.scalar.activation(h1[:], f1s[k][:], AF.Relu, bias=b1[:])
                f2 = f2_ps.tile([10, BT], f32, name=f"f2_{k}", tag="f2")
                nc.tensor.matmul(f2[:], w2[:], h1[:], start=True, stop=True)
                nc.scalar.activation(y_sb[:, it * BT:(it + 1) * BT], f2[:],
                                     AF.Identity, bias=b2[:])

        # ---- single store at the very end (feature-major; host transposes)
        nc.sync.dma_start(y_d, y_sb[:])

    nc.compile()
    return nc


def _host_prep_v4(inputs):
    import ml_dtypes
    bf = ml_dtypes.bfloat16
    x = np.asarray(inputs["x"], dtype=np.float32)
    w = np.asarray(inputs["conv_w"], dtype=np.float32)
    fc1_w = np.asarray(inputs["fc1_w"], dtype=np.float32)
    fc1_b = np.asarray(inputs["fc1_b"], dtype=np.float32)
    fc2_w = np.asarray(inputs["fc2_w"], dtype=np.float32)
    fc2_b = np.asarray(inputs["fc2_b"], dtype=np.float32)

    xp = np.ascontiguousarray(x.astype(bf))

    kA = np.zeros((112, 104), np.float32)
    kB = np.zeros((56, 104), np.float32)
    kC = np.zeros((112, 52), np.float32)
    for oi in range(4):
        for oj in range(26):
            m = oi * 26 + oj
            for di in range(3):
                for dj in range(3):
                    ri, ci = oi + di, oj + dj
                    if ri < 4:
                        kA[ri * 28 + ci, m] = w[di, dj]
                    else:
                        kB[(ri - 4) * 28 + ci, m] = w[di, dj]
    for oi in range(2):
        for oj in range(26):
            m = oi * 26 + oj
            for di in range(3):
                for dj in range(3):
                    kC[(oi + di) * 28 + (oj + dj), m] = w[di, dj]

    consts = {
        "kA": kA.astype(bf),
        "kB": kB.astype(bf),
        "kC": kC.astype(bf),
        "iden": np.eye(128, dtype=np.float32).astype(bf),
        "w1": np.ascontiguousarray(fc1_w.T),
        "b1": np.ascontiguousarray(fc1_b.reshape(128, 1)),
        "w2": np.ascontiguousarray(fc2_w.T),
        "b2": np.ascontiguousarray(fc2_b.reshape(10, 1)),
    }
    in_maps = []
    for c in range(N_CORES):
        m = {"x": xp[c * B_CORE:(c + 1) * B_CORE]}
        m.update(consts)
        in_maps.append(m)
    return in_maps



def _build_module_v6(b_core=B_CORE, n_cores=N_CORES):
    import concourse.bass as bass
    import concourse.tile as tile
    from concourse import bacc, mybir

    f32 = mybir.dt.float32
    f32r = mybir.dt.float32r
    bf16 = mybir.dt.bfloat16
    AF = mybir.ActivationFunctionType
    nt = b_core // BT

    nc = bacc.Bacc("TRN2", target_bir_lowering=False, debug=False,
                   num_devices=n_cores)

    # x arrives feature-major from the host: [7 row-groups, 112 pixels, batch]
    x_d = nc.dram_tensor("x", [7, 112, b_core], bf16, kind="ExternalInput").ap()
    kA_d = nc.dram_tensor("kA", [112, 104], bf16, kind="ExternalInput").ap()
    kB_d = nc.dram_tensor("kB", [56, 104], bf16, kind="ExternalInput").ap()
    kC_d = nc.dram_tensor("kC", [112, 52], bf16, kind="ExternalInput").ap()
    w1_d = nc.dram_tensor("w1", [676, 128], bf16, kind="ExternalInput").ap()
    b1_d = nc.dram_tensor("b1", [128, 1], f32, kind="ExternalInput").ap()
    w2_d = nc.dram_tensor("w2", [128, 10], bf16, kind="ExternalInput").ap()
    b2_d = nc.dram_tensor("b2", [10, 1], f32, kind="ExternalInput").ap()
    y_d = nc.dram_tensor("y", [10, b_core], f32, kind="ExternalOutput").ap()

    with tile.TileContext(nc) as tc, ExitStack() as ctx:
        const = ctx.enter_context(tc.tile_pool(name="const", bufs=1))
        xfm_p = ctx.enter_context(tc.tile_pool(name="xfm", bufs=3))
        h_p = ctx.enter_context(tc.tile_pool(name="h", bufs=14))
        h1_p = ctx.enter_context(tc.tile_pool(name="h1", bufs=2))
        o_p = ctx.enter_context(tc.tile_pool(name="osb", bufs=1))
        cv_ps = ctx.enter_context(tc.tile_pool(name="cv_ps", bufs=4, space="PSUM"))
        f1_ps = ctx.enter_context(tc.tile_pool(name="f1_ps", bufs=2, space="PSUM"))
        f2_ps = ctx.enter_context(tc.tile_pool(name="f2_ps", bufs=2, space="PSUM"))

        kA = const.tile([112, 104], bf16, name="kA")
        nc.sync.dma_start(kA[:], kA_d)
        kB = const.tile([56, 104], bf16, name="kB")
        nc.sync.dma_start(kB[:], kB_d)
        kC = const.tile([112, 52], bf16, name="kC")
        nc.sync.dma_start(kC[:], kC_d)

        w1 = []
        offs = 0
        for b in range(7):
            m = 104 if b < 6 else 52
            t = const.tile([m, 128], bf16, tag=f"w1_{b}", name=f"w1_{b}")
            nc.sync.dma_start(t[:], w1_d[offs:offs + m, :])
            w1.append(t)
            offs += m
        w2 = const.tile([128, 10], bf16, name="w2")
        nc.sync.dma_start(w2[:], w2_d)
        b1 = const.tile([128, 1], f32, name="b1")
        nc.sync.dma_start(b1[:], b1_d)
        b2 = const.tile([10, 1], f32, name="b2")
        nc.sync.dma_start(b2[:], b2_d)

        y_sb = o_p.tile([10, b_core], f32, name="y_sb")

        # Two batch-tiles are processed as interleaved instruction streams:
        # consecutive PE matmuls belong to independent tiles (different PSUM
        # banks, independent deps) so fill/drain phases overlap.
        def emit_pair(its):
            xfms, hss, f1s, h1s = [], [], [], []
            for k, it in enumerate(its):
                xfm = xfm_p.tile([112, 7, BT], bf16, name="xfm", tag="xfm")
                src = x_d[:, :, it * BT:(it + 1) * BT].rearrange("g p b -> p g b")
                (nc.sync if it % 2 == 0 else nc.scalar).dma_start(xfm[:], src)
                xfms.append(xfm)
                hss.append([])

            for b in range(7):
                cvs = []
                if b < 6:
                    for k in range(len(its)):
                        cv = cv_ps.tile([104, BT], f32, name="cv", tag="cv")
                        nc.tensor.matmul(cv[:], kA[:], xfms[k][:, b, :],
                                         start=True, stop=False)
                        cvs.append(cv)
                    for k in range(len(its)):
                        nc.tensor.matmul(cvs[k][:], kB[:],
                                         xfms[k][0:56, b + 1, :],
                                         start=False, stop=True)
                else:
                    for k in range(len(its)):
                        cv = cv_ps.tile([52, BT], f32, name="cv6", tag="cv")
                        nc.tensor.matmul(cv[:], kC[:], xfms[k][:, 6, :],
                                         start=True, stop=True)
                        cvs.append(cv)
                for k in range(len(its)):
                    h = h_p.tile([104 if b < 6 else 52, BT], bf16, tag="h",
                                 name=f"h{b}_{k}")
                    if (b + k) % 2 == 0:
                        nc.vector.tensor_scalar_max(h[:], cvs[k][:], 0.0)
                    else:
                        nc.scalar.activation(h[:], cvs[k][:], AF.Relu)
                    hss[k].append(h)

            for k, it in enumerate(its):
                f1s.append(f1_ps.tile([128, BT], f32, name=f"f1_{k}", tag="f1"))
            for b in range(7):
                for k in range(len(its)):
                    nc.tensor.matmul(f1s[k][:], w1[b][:], hss[k][b][:],
                                     start=(b == 0), stop=(b == 6))
            for k, it in enumerate(its):
                h1 = h1_p.tile([128, BT], bf16, name=f"h1_{k}", tag="h1")
                nc.scalar.activation(h1[:], f1s[k][:], AF.Relu, bias=b1[:])
                h1s.append(h1)
            for k, it in enumerate(its):
                f2 = f2_ps.tile([10, BT], f32, name=f"f2_{k}", tag="f2")
                nc.tensor.matmul(f2[:], w2[:], h1s[k][:], start=True, stop=True)
                nc.scalar.activation(y_sb[:, it * BT:(it + 1) * BT], f2[:],
                                     AF.Identity, bias=b2[:])

        for it2 in range(0, nt, 2):
            emit_pair([it2, it2 + 1])

        # ---- single store at the very end (feature-major; host transposes)
        nc.sync.dma_start(y_d, y_sb[:])

    nc.compile()
    return nc


def _host_prep_v6(inputs):
    import ml_dtypes
    bf = ml_dtypes.bfloat16
    x = np.asarray(inputs["x"], dtype=np.float32)
    w = np.asarray(inputs["conv_w"], dtype=np.float32)
    fc1_w = np.asarray(inputs["fc1_w"], dtype=np.float32)
    fc1_b = np.asarray(inputs["fc1_b"], dtype=np.float32)
    fc2_w = np.asarray(inputs["fc2_w"], dtype=np.float32)
    fc2_b = np.asarray(inputs["fc2_b"], dtype=np.float32)

    B = x.shape[0]
    # feature-major: [7 row-groups, 112 pixels, B]
    xT = np.ascontiguousarray(x.astype(bf).reshape(B, 7, 112).transpose(1, 2, 0))

    kA = np.zeros((112, 104), np.float32)
    kB = np.zeros((56, 104), np.float32)
    kC = np.zeros((112, 52), np.float32)
    for oi in range(4):
        for oj in range(26):
            m = oi * 26 + oj
            for di in range(3):
                for dj in range(3):
                    ri, ci = oi + di, oj + dj
                    if ri < 4:
                        kA[ri * 28 + ci, m] = w[di, dj]
                    else:
                        kB[(ri - 4) * 28 + ci, m] = w[di, dj]
    for oi in range(2):
        for oj in range(26):
            m = oi * 26 + oj
            for di in range(3):
                for dj in range(3):
                    kC[(oi + di) * 28 + (oj + dj), m] = w[di, dj]

    consts = {
        "kA": kA.astype(bf),
        "kB": kB.astype(bf),
        "kC": kC.astype(bf),
        "w1": np.ascontiguousarray(fc1_w.T.astype(bf)),
        "b1": np.ascontiguousarray(fc1_b.reshape(128, 1)),
        "w2": np.ascontiguousarray(fc2_w.T.astype(bf)),
        "b2": np.ascontiguousarray(fc2_b.reshape(10, 1)),
    }
    in_maps = []
    for c in range(N_CORES):
        m = {"x": np.ascontiguousarray(xT[:, :, c * B_CORE:(c + 1) * B_CORE])}
        m.update(consts)
        in_maps.append(m)
    return in_maps


# ---------------------------------------------------------------------------
# v8: misaligned 128-out conv groups (11 conv MMs), dense fc1 (6 MMs), fc2 (1)
# = 18 matmuls per 512-batch tile.  Feature chunks are DMA'd pre-gathered and
# zero-padded so every stationary is [128, 128] (FWL-eligible) and every
# moving operand is a contiguous [128, 512] slice.  Software-pipelined skew:
# PE stream per tile = [fc2(t-2)] [conv(t) x11] [fc1(t-1) x6].
# ---------------------------------------------------------------------------

# chunk -> (pixel start, valid size); 2 chunks per 128-out group, 1 for the
# final 36-out group.  Receptive rows per group: g covers out rows
# [o//26 .. (o+m-1)//26 + 2] -> pixel span split into <=128-partition chunks.
V8_CHUNKS = [(0, 128), (128, 68), (112, 128), (240, 96), (252, 128), (380, 96),
             (392, 128), (520, 96), (532, 128), (660, 96), (672, 112)]
V8_CHUNK_GROUP = [0, 0, 1, 1, 2, 2, 3, 3, 4, 4, 5]
V8_GROUP_OUT = [(0, 128), (128, 128), (256, 128), (384, 128), (512, 128),
                (640, 36)]
V8_NCH = 11
V8_WORDS = V8_NCH * BT  # 5632 moving words per tile per partition


def _build_module_v8(b_core=B_CORE, n_cores=N_CORES, prefetch=3):
    import concourse.bass as bass
    import concourse.tile as tile
    from concourse import bacc, mybir

    f32 = mybir.dt.float32
    bf16 = mybir.dt.bfloat16
    AF = mybir.ActivationFunctionType
    nt = b_core // BT

    nc = bacc.Bacc("TRN2", target_bir_lowering=False, debug=False,
                   num_devices=n_cores)

    x_d = nc.dram_tensor("x", [nt, 128, V8_WORDS], bf16,
                         kind="ExternalInput").ap()
    # all bf16 weights in one blob: 11 conv chunks + 6 fc1 chunks ([128,128]
    # each) + fc2 [128,10] concatenated along free dim
    wb_d = nc.dram_tensor("wblob", [128, 17 * 128 + 10], bf16,
                          kind="ExternalInput").ap()
    bias_d = nc.dram_tensor("biases", [128, 2], f32, kind="ExternalInput").ap()
    y_d = nc.dram_tensor("y", [10, b_core], f32, kind="ExternalOutput").ap()

    with tile.TileContext(nc) as tc, ExitStack() as ctx:
        const = ctx.enter_context(tc.tile_pool(name="const", bufs=1))
        xg_p = ctx.enter_context(tc.tile_pool(name="xg", bufs=prefetch + 1))
        h_p = ctx.enter_context(tc.tile_pool(name="h", bufs=12))
        h1_p = ctx.enter_context(tc.tile_pool(name="h1", bufs=2))
        o_p = ctx.enter_context(tc.tile_pool(name="osb", bufs=1))
        cv_ps = ctx.enter_context(tc.tile_pool(name="cv_ps", bufs=5, space="PSUM"))
        f1_ps = ctx.enter_context(tc.tile_pool(name="f1_ps", bufs=2, space="PSUM"))
        f2_ps = ctx.enter_context(tc.tile_pool(name="f2_ps", bufs=1, space="PSUM"))

        wblob = const.tile([128, 17 * 128 + 10], bf16, name="wblob")
        nc.sync.dma_start(wblob[:], wb_d)
        biases = const.tile([128, 2], f32, name="biases")
        nc.sync.dma_start(biases[:], bias_d)
        wc = [wblob[:, c * 128:(c + 1) * 128] for c in range(V8_NCH)]
        w1 = [wblob[:, (V8_NCH + c) * 128:(V8_NCH + c + 1) * 128]
              for c in range(6)]
        w2 = wblob[:, 17 * 128:17 * 128 + 10]
        b1 = biases[:, 0:1]
        b2 = biases[0:10, 1:2]

        y_sb = o_p.tile([10, b_core], f32, name="y_sb")

        xgs = []

        def dma_x(t):
            xg = xg_p.tile([128, V8_WORDS], bf16, tag="xg", name=f"xg{t}")
            nc.sync.dma_start(xg[:], x_d[t])
            xgs.append(xg)

        hss = {}   # t -> list of 6 h tiles
        h1s = {}   # t -> h1 tile

        def emit_conv(t):
            xg = xgs.pop(0)
            hs = []
            for g in range(6):
                cv = cv_ps.tile([128, BT], f32, tag="cv", name=f"cv{g}")
                c0 = 2 * g
                if g < 5:
                    nc.tensor.matmul(cv[:], wc[c0], xg[:, c0 * BT:(c0 + 1) * BT],
                                     start=True, stop=False)
                    nc.tensor.matmul(cv[:], wc[c0 + 1],
                                     xg[:, (c0 + 1) * BT:(c0 + 2) * BT],
                                     start=False, stop=True)
                else:
                    nc.tensor.matmul(cv[:], wc[10], xg[:, 10 * BT:11 * BT],
                                     start=True, stop=True)
                h = h_p.tile([128, BT], bf16, tag="h", name=f"h{g}")
                if g % 2 == 0:
                    nc.vector.tensor_scalar_max(h[:], cv[:], 0.0)
                else:
                    nc.scalar.activation(h[:], cv[:], AF.Relu)
                hs.append(h)
            hss[t] = hs

        def emit_fc1(t):
            hs = hss.pop(t)
            f1 = f1_ps.tile([128, BT], f32, tag="f1", name="f1")
            for c in range(6):
                nc.tensor.matmul(f1[:], w1[c], hs[c][:],
                                 start=(c == 0), stop=(c == 5))
            h1 = h1_p.tile([128, BT], bf16, tag="h1", name="h1")
            nc.scalar.activation(h1[:], f1[:], AF.Relu, bias=b1)
            h1s[t] = h1

        def emit_fc2(t):
            h1 = h1s.pop(t)
            f2 = f2_ps.tile([10, BT], f32, tag="f2", name="f2")
            nc.tensor.matmul(f2[:], w2, h1[:], start=True, stop=True)
            nc.vector.tensor_scalar_add(y_sb[:, t * BT:(t + 1) * BT], f2[:], b2)

        for t in range(prefetch):
            dma_x(t)
        for t in range(nt):
            if t + prefetch < nt:
                dma_x(t + prefetch)
            if t >= 2:
                emit_fc2(t - 2)
            emit_conv(t)
            if t >= 1:
                emit_fc1(t - 1)
            # stream out finished quarter of y while compute continues
            if t % 4 == 3 and t >= 7:
                q = t // 4 - 1  # quarter fully written: tiles 4q..4q+3
                nc.sync.dma_start(y_d[:, q * 4 * BT:(q + 1) * 4 * BT],
                                  y_sb[:, q * 4 * BT:(q + 1) * 4 * BT])
        emit_fc1(nt - 1)
        emit_fc2(nt - 2)
        emit_fc2(nt - 1)
        nc.sync.dma_start(y_d[:, 12 * BT:], y_sb[:, 12 * BT:])

    nc.compile()
    return nc


def _host_prep_v8(inputs):
    import ml_dtypes
    bf = ml_dtypes.bfloat16
    x = np.asarray(inputs["x"], dtype=np.float32)
    w = np.asarray(inputs["conv_w"], dtype=np.float32)
    fc1_w = np.asarray(inputs["fc1_w"], dtype=np.float32)
    fc1_b = np.asarray(inputs["fc1_b"], dtype=np.float32)
    fc2_w = np.asarray(inputs["fc2_w"], dtype=np.float32)
    fc2_b = np.asarray(inputs["fc2_b"], dtype=np.float32)
    B = x.shape[0]
    nt = B_CORE // BT

    # gather pixels into zero-padded 128-partition chunks
    idx = np.full(V8_NCH * 128, 784, np.int64)
    for c, (s, sz) in enumerate(V8_CHUNKS):
        idx[c * 128:c * 128 + sz] = s + np.arange(sz)
    xb = np.concatenate([x.astype(bf), np.zeros((B, 1), bf)], axis=1)[:, idx]
    # [B, 11*128] -> [core, tile, partition, chunk, sample]
    xb = xb.reshape(N_CORES, nt, BT, V8_NCH, 128).transpose(0, 1, 4, 3, 2)
    xb = np.ascontiguousarray(xb).reshape(N_CORES, nt, 128, V8_WORDS)

    # conv band stationaries [11, 128, 128]
    wcs = np.zeros((V8_NCH, 128, 128), np.float32)
    p = np.arange(128)
    j = np.arange(128)
    for c, (start, size) in enumerate(V8_CHUNKS):
        g = V8_CHUNK_GROUP[c]
        ostart, osz = V8_GROUP_OUT[g]
        px = start + p
        o = ostart + j
        di = (px // 28)[:, None] - (o // 26)[None, :]
        dj = (px % 28)[:, None] - (o % 26)[None, :]
        m = ((di >= 0) & (di < 3) & (dj >= 0) & (dj < 3)
             & (p < size)[:, None] & (j < osz)[None, :])
        wcs[c][m] = w[np.clip(di, 0, 2), np.clip(dj, 0, 2)][m]

    # fc1 chunks [6, 128, 128] (rows grouped to match h layout, zero-padded)
    w1g = np.zeros((6, 128, 128), np.float32)
    for g, (os_, sz) in enumerate(V8_GROUP_OUT):
        w1g[g, :sz, :] = fc1_w.T[os_:os_ + sz, :]

    wblob = np.concatenate(
        [wcs.transpose(1, 0, 2).reshape(128, V8_NCH * 128),
         w1g.transpose(1, 0, 2).reshape(128, 6 * 128),
         fc2_w.T], axis=1).astype(bf)
    biases = np.zeros((128, 2), np.float32)
    biases[:, 0] = fc1_b
    biases[:10, 1] = fc2_b

    in_maps = []
    for c in range(N_CORES):
        in_maps.append({"x": xb[c], "wblob": wblob, "biases": biases})
    return in_maps


# ---------------------------------------------------------------------------
# v9: like v8 but x is stored as 8 overlapping 128-pixel columns (no zero
# padding, 1024 words/sample vs 1408) chosen so every conv group's receptive
# span is covered by 2 columns (1 for the last).  Stationaries zero out taps
# outside each MM's assigned pixel set (disjoint across a group's two MMs).
# Consts go on the scalar DMA queue (overlaps the first x DMA on sync), and
# 8 warmup matmuls on the weight blob flip the PE HAM clock gate to 8/8
# while the first x tile is still in flight.
# ---------------------------------------------------------------------------

V9_COLS = [0, 112, 220, 348, 392, 500, 628, 656]
V9_GROUP_COLS = [(0, 1), (1, 2), (2, 3), (4, 5), (5, 6), (7,)]
V9_NCOL = 8
V9_WORDS = V9_NCOL * BT  # 4096


def _build_module_v9(b_core=B_CORE, n_cores=N_CORES, prefetch=3, warmup=10):
    import concourse.bass as bass
    import concourse.tile as tile
    from concourse import bacc, mybir

    f32 = mybir.dt.float32
    bf16 = mybir.dt.bfloat16
    AF = mybir.ActivationFunctionType
    nt = b_core // BT
    nmm = 11  # conv matmuls per tile

    nc = bacc.Bacc("TRN2", target_bir_lowering=False, debug=False,
                   num_devices=n_cores)

    x_d = nc.dram_tensor("x", [nt, 128, V9_WORDS], bf16,
                         kind="ExternalInput").ap()
    wb_d = nc.dram_tensor("wblob", [128, (nmm + 6) * 128 + 10], bf16,
                          kind="ExternalInput").ap()
    bias_d = nc.dram_tensor("biases", [128, 2], f32, kind="ExternalInput").ap()
    y_d = nc.dram_tensor("y", [10, b_core], f32, kind="ExternalOutput").ap()

    with tile.TileContext(nc) as tc, ExitStack() as ctx:
        const = ctx.enter_context(tc.tile_pool(name="const", bufs=1))
        xg_p = ctx.enter_context(tc.tile_pool(name="xg", bufs=prefetch + 1))
        h_p = ctx.enter_context(tc.tile_pool(name="h", bufs=12))
        h1_p = ctx.enter_context(tc.tile_pool(name="h1", bufs=2))
        o_p = ctx.enter_context(tc.tile_pool(name="osb", bufs=1))
        cv_ps = ctx.enter_context(tc.tile_pool(name="cv_ps", bufs=5, space="PSUM"))
        f1_ps = ctx.enter_context(tc.tile_pool(name="f1_ps", bufs=2, space="PSUM"))
        f2_ps = ctx.enter_context(tc.tile_pool(name="f2_ps", bufs=1, space="PSUM"))

        # warm up the PE HAM clock gate on an on-chip zero tile (no DMA dep)
        warm_src = const.tile([128, BT], bf16, name="warm_src")
        nc.gpsimd.memset(warm_src[:], 0)
        for _ in range(warmup):
            wm = f1_ps.tile([128, BT], f32, tag="f1", name="warm")
            nc.tensor.matmul(wm[:], warm_src[:, 0:128], warm_src[:],
                             start=True, stop=True)

        wblob = const.tile([128, (nmm + 6) * 128 + 10], bf16, name="wblob")
        # conv weights first so the first conv matmul's dep is small
        nc.scalar.dma_start(wblob[:, :nmm * 128], wb_d[:, :nmm * 128])
        nc.scalar.dma_start(wblob[:, nmm * 128:], wb_d[:, nmm * 128:])
        biases = const.tile([128, 2], f32, name="biases")
        nc.scalar.dma_start(biases[:], bias_d)
        wc = [wblob[:, c * 128:(c + 1) * 128] for c in range(nmm)]
        w1 = [wblob[:, (nmm + c) * 128:(nmm + c + 1) * 128] for c in range(6)]
        w2 = wblob[:, (nmm + 6) * 128:(nmm + 6) * 128 + 10]
        b1 = biases[:, 0:1]
        b2 = biases[0:10, 1:2]

        y_sb = o_p.tile([10, b_core], f32, name="y_sb")

        xgs = []

        def dma_x(t, split=False):
            xg = xg_p.tile([128, V9_WORDS], bf16, tag="xg", name=f"xg{t}")
            if split:
                # conv g0 needs cols 0-1, g1 needs 1-2; rest arrive second
                for lo, hi in ((0, 3), (3, 8)):
                    nc.sync.dma_start(xg[:, lo * BT:hi * BT],
                                      x_d[t][:, lo * BT:hi * BT])
            else:
                nc.sync.dma_start(xg[:], x_d[t])
            xgs.append(xg)

        hss = {}
        h1s = {}

        def emit_conv(t):
            xg = xgs.pop(0)
            mi = 0
            hs = []
            for g in range(6):
                cols = V9_GROUP_COLS[g]
                cv = cv_ps.tile([128, BT], f32, tag="cv", name=f"cv{g}")
                for k, col in enumerate(cols):
                    nc.tensor.matmul(cv[:], wc[mi],
                                     xg[:, col * BT:(col + 1) * BT],
                                     start=(k == 0), stop=(k == len(cols) - 1))
                    mi += 1
                h = h_p.tile([128, BT], bf16, tag="h", name=f"h{g}")
                if g % 2 == 0:
                    nc.vector.tensor_scalar_max(h[:], cv[:], 0.0)
                else:
                    nc.scalar.activation(h[:], cv[:], AF.Relu)
                hs.append(h)
            hss[t] = hs

        def emit_fc1(t):
            hs = hss.pop(t)
            f1 = f1_ps.tile([128, BT], f32, tag="f1", name="f1")
            for c in range(6):
                nc.tensor.matmul(f1[:], w1[c], hs[c][:],
                                 start=(c == 0), stop=(c == 5))
            h1 = h1_p.tile([128, BT], bf16, tag="h1", name="h1")
            nc.scalar.activation(h1[:], f1[:], AF.Relu, bias=b1)
            h1s[t] = h1

        def emit_fc2(t):
            h1 = h1s.pop(t)
            f2 = f2_ps.tile([10, BT], f32, tag="f2", name="f2")
            nc.tensor.matmul(f2[:], w2, h1[:], start=True, stop=True)
            nc.vector.tensor_scalar_add(y_sb[:, t * BT:(t + 1) * BT], f2[:], b2)

        for t in range(prefetch):
            dma_x(t, split=(t == 0))
        for t in range(nt):
            if t + prefetch < nt:
                dma_x(t + prefetch)
            if t >= 2:
                emit_fc2(t - 2)
            emit_conv(t)
            if t >= 1:
                emit_fc1(t - 1)
            if t % 4 == 3 and t >= 7:
                q = t // 4 - 1
                nc.sync.dma_start(y_d[:, q * 4 * BT:(q + 1) * 4 * BT],
                                  y_sb[:, q * 4 * BT:(q + 1) * 4 * BT])
        emit_fc1(nt - 1)
        emit_fc2(nt - 2)
        nc.sync.dma_start(y_d[:, 12 * BT:15 * BT], y_sb[:, 12 * BT:15 * BT])
        emit_fc2(nt - 1)
        nc.sync.dma_start(y_d[:, 15 * BT:], y_sb[:, 15 * BT:])

    nc.compile()
    return nc


def _host_prep_v9(inputs):
    import ml_dtypes
    bf = ml_dtypes.bfloat16
    x = np.asarray(inputs["x"], dtype=np.float32)
    w = np.asarray(inputs["conv_w"], dtype=np.float32)
    fc1_w = np.asarray(inputs["fc1_w"], dtype=np.float32)
    fc1_b = np.asarray(inputs["fc1_b"], dtype=np.float32)
    fc2_w = np.asarray(inputs["fc2_w"], dtype=np.float32)
    fc2_b = np.asarray(inputs["fc2_b"], dtype=np.float32)
    B = x.shape[0]
    nt = B_CORE // BT

    idx = np.concatenate([np.arange(a, a + 128) for a in V9_COLS])
    xb = x.astype(bf)[:, idx]
    xb = xb.reshape(N_CORES, nt, BT, V9_NCOL, 128).transpose(0, 1, 4, 3, 2)
    xb = np.ascontiguousarray(xb).reshape(N_CORES, nt, 128, V9_WORDS)

    # conv stationaries: per group, taps assigned to the first column that
    # contains the pixel (disjoint coverage across the group's matmuls)
    p = np.arange(128)
    j = np.arange(128)
    wcs = []
    for g, cols in enumerate(V9_GROUP_COLS):
        ostart, osz = V8_GROUP_OUT[g]
        covered_lo = None  # pixel range already handled by earlier col
        for col in cols:
            a = V9_COLS[col]
            px = a + p
            o = ostart + j
            di = (px // 28)[:, None] - (o // 26)[None, :]
            dj = (px % 28)[:, None] - (o % 26)[None, :]
            m = ((di >= 0) & (di < 3) & (dj >= 0) & (dj < 3)
                 & (j < osz)[None, :])
            if covered_lo is not None:
                lo, hi = covered_lo
                m &= ~((px >= lo) & (px < hi))[:, None]
            W = np.zeros((128, 128), np.float32)
            W[m] = w[np.clip(di, 0, 2), np.clip(dj, 0, 2)][m]
            wcs.append(W)
            covered_lo = (a, a + 128) if covered_lo is None else \
                (min(covered_lo[0], a), max(covered_lo[1], a + 128))
    wcs = np.stack(wcs)  # [11, 128, 128]

    w1g = np.zeros((6, 128, 128), np.float32)
    for g, (os_, sz) in enumerate(V8_GROUP_OUT):
        w1g[g, :sz, :] = fc1_w.T[os_:os_ + sz, :]

    wblob = np.concatenate(
        [wcs.transpose(1, 0, 2).reshape(128, 11 * 128),
         w1g.transpose(1, 0, 2).reshape(128, 6 * 128),
         fc2_w.T], axis=1).astype(bf)
    biases = np.zeros((128, 2), np.float32)
    biases[:, 0] = fc1_b
    biases[:10, 1] = fc2_b

    in_maps = []
    for c in range(N_CORES):
        in_maps.append({"x": xb[c], "wblob": wblob, "biases": biases})
    return in_maps


VERSION = 9


def run(inputs, trace=False, tmpdir=None, version=None):
    from concourse.bass_utils import run_bass_kernel_spmd

    version = VERSION if version is None else version
    key = f"nc{version}"
    builders = {9: _build_module_v9, 8: _build_module_v8, 6: _build_module_v6,
                4: _build_module_v4, 2: _build_module}
    preps = {9: _host_prep_v9, 8: _host_prep_v8, 6: _host_prep_v6,
             4: _host_prep_v4, 2: _host_prep}
    if key not in _cache:
        _cache[key] = builders[version]()
    nc = _cache[key]
    in_maps = preps[version](inputs)
    res = run_bass_kernel_spmd(nc, in_maps, list(range(N_CORES)), trace=trace,
                               tmpdir=tmpdir)
    out = np.concatenate([np.ascontiguousarray(r["y"].T) for r in res.results], axis=0)
    return out, res


def kernel(**inputs) -> np.ndarray:
    out, _ = run(inputs, trace=False)
    return out



# revision 22
# speedup vs baseline: 1.0326x; 1.0042x over previous
"""Trainium2 Bass kernel for DigitConvolutionalModel.

Pipeline (per core, pure data-parallel over batch):
  x [8192, 784] --DMA--> SBUF batch-major --PE transpose--> feature-major tiles
  conv 3x3 as banded block-matmuls on PE -> relu -> fc1 (matmul) -> relu
  -> fc2 (matmul) + bias -> DMA out.

All activations live feature-major ([features, batch]) so the PE can contract
over the partition dim. The 3x3 conv is expressed as 13 small banded matmuls
per 512-batch tile using three constant band matrices built on the host from
conv_w (shift-invariant across 4-image-row blocks).
"""

import numpy as np
from contextlib import ExitStack

N_CORES = 8
B_FULL = 65536
B_CORE = B_FULL // N_CORES  # 8192
BT = 512                    # batch tile (matmul moving free dim)
NT = B_CORE // BT           # 16

_cache = {}


def _build_module(b_core=B_CORE, n_cores=N_CORES):
    import concourse.bass as bass
    import concourse.tile as tile
    from concourse import bacc, mybir

    f32 = mybir.dt.float32
    f32r = mybir.dt.float32r
    AF = mybir.ActivationFunctionType
    nt = b_core // BT

    nc = bacc.Bacc("TRN2", target_bir_lowering=False, debug=False,
                   num_devices=n_cores)

    x_d = nc.dram_tensor("x", [b_core, 784], f32, kind="ExternalInput").ap()
    kA_d = nc.dram_tensor("kA", [112, 104], f32, kind="ExternalInput").ap()
    kB_d = nc.dram_tensor("kB", [56, 104], f32, kind="ExternalInput").ap()
    kC_d = nc.dram_tensor("kC", [112, 52], f32, kind="ExternalInput").ap()
    w1_d = nc.dram_tensor("w1", [676, 128], f32, kind="ExternalInput").ap()
    b1_d = nc.dram_tensor("b1", [128, 1], f32, kind="ExternalInput").ap()
    w2_d = nc.dram_tensor("w2", [128, 10], f32, kind="ExternalInput").ap()
    b2_d = nc.dram_tensor("b2", [10, 1], f32, kind="ExternalInput").ap()
    id_d = nc.dram_tensor("iden", [128, 128], f32, kind="ExternalInput").ap()
    y_d = nc.dram_tensor("y", [10, b_core], f32, kind="ExternalOutput").ap()

    with tile.TileContext(nc) as tc, ExitStack() as ctx:
        const = ctx.enter_context(tc.tile_pool(name="const", bufs=1))
        xbm_p = ctx.enter_context(tc.tile_pool(name="xbm", bufs=4))
        xfm_p = ctx.enter_context(tc.tile_pool(name="xfm", bufs=21))
        h_p = ctx.enter_context(tc.tile_pool(name="h", bufs=14))
        h1_p = ctx.enter_context(tc.tile_pool(name="h1", bufs=2))
        o_p = ctx.enter_context(tc.tile_pool(name="osb", bufs=2))
        tp_ps = ctx.enter_context(tc.tile_pool(name="tp_ps", bufs=2, space="PSUM"))
        cv_ps = ctx.enter_context(tc.tile_pool(name="cv_ps", bufs=2, space="PSUM"))
        f1_ps = ctx.enter_context(tc.tile_pool(name="f1_ps", bufs=2, space="PSUM"))
        f2_ps = ctx.enter_context(tc.tile_pool(name="f2_ps", bufs=2, space="PSUM"))

        iden = const.tile([128, 128], f32, name="iden")
        nc.sync.dma_start(iden[:], id_d)

        def load_f32r(name, shape, src):
            stg = const.tile(shape, f32, tag=f"{name}_stg", name=f"{name}_stg")
            nc.sync.dma_start(stg[:], src)
            t = const.tile(shape, f32r, tag=name, name=name)
            nc.vector.tensor_copy(t[:], stg[:])
            return t

        kA = load_f32r("kA", [112, 104], kA_d)
        kB = load_f32r("kB", [56, 104], kB_d)
        kC = load_f32r("kC", [112, 52], kC_d)
        w1 = []
        offs = 0
        for b in range(7):
            m = 104 if b < 6 else 52
            w1.append(load_f32r(f"w1_{b}", [m, 128], w1_d[offs:offs + m, :]))
            offs += m
        w2 = load_f32r("w2", [128, 10], w2_d)
        b1 = const.tile([128, 1], f32, name="b1")
        nc.sync.dma_start(b1[:], b1_d)
        b2 = const.tile([10, 1], f32, name="b2")
        nc.sync.dma_start(b2[:], b2_d)

        for it in range(nt):
            # ---- load one batch tile, batch-major [128, 4, 784]
            xbm = xbm_p.tile([128, 4, 784], f32, name="xbm", tag="xbm")
            src = x_d[it * BT:(it + 1) * BT, :].rearrange("(c p) d -> p c d", p=128)
            (nc.sync if it % 2 == 0 else nc.scalar).dma_start(xbm[:], src)

            # ---- transpose to feature-major tiles xfm[t] = x.T rows 112t..112t+111
            xfm = []
            for t in range(7):
                tp = tp_ps.tile([112, BT], f32, name="tp", tag="tp")
                for c in range(4):
                    nc.tensor.transpose(tp[:, c * 128:(c + 1) * 128],
                                        xbm[:, c, 112 * t:112 * t + 112],
                                        iden[:])
                xf = xfm_p.tile([112, BT], f32r, tag="xfm", name=f"xfm{t}")
                if t % 2 == 0:
                    nc.vector.tensor_copy(xf[:], tp[:])
                else:
                    nc.scalar.copy(xf[:], tp[:])
                xfm.append(xf)

            # ---- conv as banded matmuls, relu into h blocks
            hs = []
            for b in range(6):
                cv = cv_ps.tile([104, BT], f32, name="cv", tag="cv")
                nc.tensor.matmul(cv[:], kA[:], xfm[b][:], start=True, stop=False)
                nc.tensor.matmul(cv[:], kB[:], xfm[b + 1][0:56, :],
                                 start=False, stop=True)
                h = h_p.tile([104, BT], f32r, tag="h", name=f"h{b}")
                if b % 2 == 0:
                    nc.vector.tensor_scalar_max(h[:], cv[:], 0.0)
                else:
                    nc.scalar.activation(h[:], cv[:], AF.Relu)
                hs.append(h)
            cv = cv_ps.tile([52, BT], f32, name="cv6", tag="cv")
            nc.tensor.matmul(cv[:], kC[:], xfm[6][:], start=True, stop=True)
            h = h_p.tile([52, BT], f32r, tag="h", name="h6")
            nc.vector.tensor_scalar_max(h[:], cv[:], 0.0)
            hs.append(h)

            # ---- fc1: accumulate 7 chunks, relu + bias
            f1 = f1_ps.tile([128, BT], f32, name="f1", tag="f1")
            for b in range(7):
                nc.tensor.matmul(f1[:], w1[b][:], hs[b][:],
                                 start=(b == 0), stop=(b == 6))
            h1 = h1_p.tile([128, BT], f32r, name="h1", tag="h1")
            nc.scalar.activation(h1[:], f1[:], AF.Relu, bias=b1[:])

            # ---- fc2 + bias
            f2 = f2_ps.tile([10, BT], f32, name="f2", tag="f2")
            nc.tensor.matmul(f2[:], w2[:], h1[:], start=True, stop=True)
            osb = o_p.tile([10, BT], f32, name="osb", tag="osb")
            nc.scalar.activation(osb[:], f2[:], AF.Identity, bias=b2[:])

            # ---- store (feature-major; host transposes)
            nc.sync.dma_start(y_d[:, it * BT:(it + 1) * BT], osb[:])

    nc.compile()
    return nc


def _host_prep(inputs):
    x = np.ascontiguousarray(np.asarray(inputs["x"], dtype=np.float32))
    w = np.asarray(inputs["conv_w"], dtype=np.float32)
    fc1_w = np.asarray(inputs["fc1_w"], dtype=np.float32)
    fc1_b = np.asarray(inputs["fc1_b"], dtype=np.float32)
    fc2_w = np.asarray(inputs["fc2_w"], dtype=np.float32)
    fc2_b = np.asarray(inputs["fc2_b"], dtype=np.float32)

    kA = np.zeros((112, 104), np.float32)
    kB = np.zeros((56, 104), np.float32)
    kC = np.zeros((112, 52), np.float32)
    for oi in range(4):
        for oj in range(26):
            m = oi * 26 + oj
            for di in range(3):
                for dj in range(3):
                    ri, ci = oi + di, oj + dj
                    if ri < 4:
                        kA[ri * 28 + ci, m] = w[di, dj]
                    else:
                        kB[(ri - 4) * 28 + ci, m] = w[di, dj]
    for oi in range(2):
        for oj in range(26):
            m = oi * 26 + oj
            for di in range(3):
                for dj in range(3):
                    kC[(oi + di) * 28 + (oj + dj), m] = w[di, dj]

    consts = {
        "kA": kA,
        "kB": kB,
        "kC": kC,
        "w1": np.ascontiguousarray(fc1_w.T),
        "b1": np.ascontiguousarray(fc1_b.reshape(128, 1)),
        "w2": np.ascontiguousarray(fc2_w.T),
        "b2": np.ascontiguousarray(fc2_b.reshape(10, 1)),
        "iden": np.eye(128, dtype=np.float32),
    }
    in_maps = []
    for c in range(N_CORES):
        m = {"x": x[c * B_CORE:(c + 1) * B_CORE]}
        m.update(consts)
        in_maps.append(m)
    return in_maps


GBT = 2048                  # batch rows per DMA-transpose group (4 tiles)


def _build_module_v4(b_core=B_CORE, n_cores=N_CORES):
    import concourse.bass as bass
    import concourse.tile as tile
    from concourse import bacc, mybir

    f32 = mybir.dt.float32
    f32r = mybir.dt.float32r
    bf16 = mybir.dt.bfloat16
    AF = mybir.ActivationFunctionType
    nt = b_core // BT

    nc = bacc.Bacc("TRN2", target_bir_lowering=False, debug=False,
                   num_devices=n_cores)

    x_d = nc.dram_tensor("x", [b_core, 784], bf16, kind="ExternalInput").ap()
    kA_d = nc.dram_tensor("kA", [112, 104], bf16, kind="ExternalInput").ap()
    kB_d = nc.dram_tensor("kB", [56, 104], bf16, kind="ExternalInput").ap()
    kC_d = nc.dram_tensor("kC", [112, 52], bf16, kind="ExternalInput").ap()
    id_d = nc.dram_tensor("iden", [128, 128], bf16, kind="ExternalInput").ap()
    w1_d = nc.dram_tensor("w1", [676, 128], f32, kind="ExternalInput").ap()
    b1_d = nc.dram_tensor("b1", [128, 1], f32, kind="ExternalInput").ap()
    w2_d = nc.dram_tensor("w2", [128, 10], f32, kind="ExternalInput").ap()
    b2_d = nc.dram_tensor("b2", [10, 1], f32, kind="ExternalInput").ap()
    y_d = nc.dram_tensor("y", [10, b_core], f32, kind="ExternalOutput").ap()

    with tile.TileContext(nc) as tc, ExitStack() as ctx:
        const = ctx.enter_context(tc.tile_pool(name="const", bufs=1))
        xbm_p = ctx.enter_context(tc.tile_pool(name="xbm", bufs=4))
        xfm_p = ctx.enter_context(tc.tile_pool(name="xfm", bufs=21))
        h_p = ctx.enter_context(tc.tile_pool(name="h", bufs=14))
        h1_p = ctx.enter_context(tc.tile_pool(name="h1", bufs=2))
        o_p = ctx.enter_context(tc.tile_pool(name="osb", bufs=1))
        tp_ps = ctx.enter_context(tc.tile_pool(name="tp_ps", bufs=2, space="PSUM"))
        cv_ps = ctx.enter_context(tc.tile_pool(name="cv_ps", bufs=3, space="PSUM"))
        f1_ps = ctx.enter_context(tc.tile_pool(name="f1_ps", bufs=2, space="PSUM"))
        f2_ps = ctx.enter_context(tc.tile_pool(name="f2_ps", bufs=1, space="PSUM"))

        iden = const.tile([128, 128], bf16, name="iden")
        nc.sync.dma_start(iden[:], id_d)
        kA = const.tile([112, 104], bf16, name="kA")
        nc.sync.dma_start(kA[:], kA_d)
        kB = const.tile([56, 104], bf16, name="kB")
        nc.sync.dma_start(kB[:], kB_d)
        kC = const.tile([112, 52], bf16, name="kC")
        nc.sync.dma_start(kC[:], kC_d)

        def load_f32r(name, shape, src):
            stg = const.tile(shape, f32, tag=f"{name}_stg", name=f"{name}_stg")
            nc.sync.dma_start(stg[:], src)
            t = const.tile(shape, f32r, tag=name, name=name)
            nc.vector.tensor_copy(t[:], stg[:])
            return t

        w1 = []
        offs = 0
        for b in range(7):
            m = 104 if b < 6 else 52
            w1.append(load_f32r(f"w1_{b}", [m, 128], w1_d[offs:offs + m, :]))
            offs += m
        w2 = load_f32r("w2", [128, 10], w2_d)
        b1 = const.tile([128, 1], f32, name="b1")
        nc.sync.dma_start(b1[:], b1_d)
        b2 = const.tile([10, 1], f32, name="b2")
        nc.sync.dma_start(b2[:], b2_d)

        y_sb = o_p.tile([10, b_core], f32, name="y_sb")

        # Phase-separated pairs: all transposes for two batch tiles, then one
        # dense block of 42 real matmuls so the PE clock-gate releases.
        def tp_group(xbm, xfm, t, eng):
            tp = tp_ps.tile([112, BT], bf16, name="tp", tag="tp")
            for c in range(4):
                nc.tensor.transpose(tp[:, c * 128:(c + 1) * 128],
                                    xbm[:, c, 112 * t:112 * t + 112],
                                    iden[:])
            xf = xfm_p.tile([112, BT], bf16, tag="xfm", name=f"xfm{t}")
            if eng == 0:
                nc.vector.tensor_copy(xf[:], tp[:])
            else:
                nc.scalar.copy(xf[:], tp[:])
            xfm.append(xf)

        def conv_block(xfm, hs, b, eng):
            if b < 6:
                cv = cv_ps.tile([104, BT], f32, name="cv", tag="cv")
                nc.tensor.matmul(cv[:], kA[:], xfm[b][:], start=True, stop=False)
                nc.tensor.matmul(cv[:], kB[:], xfm[b + 1][0:56, :],
                                 start=False, stop=True)
                h = h_p.tile([104, BT], f32r, tag="h", name=f"h{b}")
            else:
                cv = cv_ps.tile([52, BT], f32, name="cv6", tag="cv")
                nc.tensor.matmul(cv[:], kC[:], xfm[6][:], start=True, stop=True)
                h = h_p.tile([52, BT], f32r, tag="h", name="h6")
            if eng == 0:
                nc.vector.tensor_scalar_max(h[:], cv[:], 0.0)
            else:
                nc.scalar.activation(h[:], cv[:], AF.Relu)
            hs.append(h)

        for it0 in range(0, nt, 2):
            its = [it0, it0 + 1]
            xfms = []
            hss = []
            for k, it in enumerate(its):
                xbm = xbm_p.tile([128, 4, 784], bf16, name="xbm", tag="xbm")
                src = x_d[it * BT:(it + 1) * BT, :].rearrange(
                    "(c p) d -> p c d", p=128)
                (nc.sync if it % 2 == 0 else nc.scalar).dma_start(xbm[:], src)
                xfm = []
                for t in range(7):
                    tp_group(xbm, xfm, t, (t + k) % 2)
                xfms.append(xfm)
                hss.append([])

            for k in range(2):
                for b in range(7):
                    conv_block(xfms[k], hss[k], b, (b + k) % 2)

            f1s = []
            for k, it in enumerate(its):
                f1s.append(f1_ps.tile([128, BT], f32, name=f"f1_{k}", tag="f1"))
            for b in range(7):
                for k in range(2):
                    nc.tensor.matmul(f1s[k][:], w1[b][:], hss[k][b][:],
                                     start=(b == 0), stop=(b == 6))
            for k, it in enumerate(its):
                h1 = h1_p.tile([128, BT], f32r, name=f"h1_{k}", tag="h1")
                nc.scalar.activation(h1[:], f1s[k][:], AF.Relu, bias=b1[:])
                f2 = f2_ps.tile([10, BT], f32, name=f"f2_{k}", tag="f2")
                nc.tensor.matmul(f2[:], w2[:], h1[:], start=True, stop=True)
                nc.scalar.activation(y_sb[:, it * BT:(it + 1) * BT], f2[:],
                                     AF.Identity, bias=b2[:])

        # ---- single store at the very end (feature-major; host transposes)
        nc.sync.dma_start(y_d, y_sb[:])

    nc.compile()
    return nc


def _host_prep_v4(inputs):
    import ml_dtypes
    bf = ml_dtypes.bfloat16
    x = np.asarray(inputs["x"], dtype=np.float32)
    w = np.asarray(inputs["conv_w"], dtype=np.float32)
    fc1_w = np.asarray(inputs["fc1_w"], dtype=np.float32)
    fc1_b = np.asarray(inputs["fc1_b"], dtype=np.float32)
    fc2_w = np.asarray(inputs["fc2_w"], dtype=np.float32)
    fc2_b = np.asarray(inputs["fc2_b"], dtype=np.float32)

    xp = np.ascontiguousarray(x.astype(bf))

    kA = np.zeros((112, 104), np.float32)
    kB = np.zeros((56, 104), np.float32)
    kC = np.zeros((112, 52), np.float32)
    for oi in range(4):
        for oj in range(26):
            m = oi * 26 + oj
            for di in range(3):
                for dj in range(3):
                    ri, ci = oi + di, oj + dj
                    if ri < 4:
                        kA[ri * 28 + ci, m] = w[di, dj]
                    else:
                        kB[(ri - 4) * 28 + ci, m] = w[di, dj]
    for oi in range(2):
        for oj in range(26):
            m = oi * 26 + oj
            for di in range(3):
                for dj in range(3):
                    kC[(oi + di) * 28 + (oj + dj), m] = w[di, dj]

    consts = {
        "kA": kA.astype(bf),
        "kB": kB.astype(bf),
        "kC": kC.astype(bf),
        "iden": np.eye(128, dtype=np.float32).astype(bf),
        "w1": np.ascontiguousarray(fc1_w.T),
        "b1": np.ascontiguousarray(fc1_b.reshape(128, 1)),
        "w2": np.ascontiguousarray(fc2_w.T),
        "b2": np.ascontiguousarray(fc2_b.reshape(10, 1)),
    }
    in_maps = []
    for c in range(N_CORES):
        m = {"x": xp[c * B_CORE:(c + 1) * B_CORE]}
        m.update(consts)
        in_maps.append(m)
    return in_maps



def _build_module_v6(b_core=B_CORE, n_cores=N_CORES):
    import concourse.bass as bass
    import concourse.tile as tile
    from concourse import bacc, mybir

    f32 = mybir.dt.float32
    f32r = mybir.dt.float32r
    bf16 = mybir.dt.bfloat16
    AF = mybir.ActivationFunctionType
    nt = b_core // BT

    nc = bacc.Bacc("TRN2", target_bir_lowering=False, debug=False,
                   num_devices=n_cores)

    # x arrives feature-major from the host: [7 row-groups, 112 pixels, batch]
    x_d = nc.dram_tensor("x", [7, 112, b_core], bf16, kind="ExternalInput").ap()
    kA_d = nc.dram_tensor("kA", [112, 104], bf16, kind="ExternalInput").ap()
    kB_d = nc.dram_tensor("kB", [56, 104], bf16, kind="ExternalInput").ap()
    kC_d = nc.dram_tensor("kC", [112, 52], bf16, kind="ExternalInput").ap()
    w1_d = nc.dram_tensor("w1", [676, 128], bf16, kind="ExternalInput").ap()
    b1_d = nc.dram_tensor("b1", [128, 1], f32, kind="ExternalInput").ap()
    w2_d = nc.dram_tensor("w2", [128, 10], bf16, kind="ExternalInput").ap()
    b2_d = nc.dram_tensor("b2", [10, 1], f32, kind="ExternalInput").ap()
    y_d = nc.dram_tensor("y", [10, b_core], f32, kind="ExternalOutput").ap()

    with tile.TileContext(nc) as tc, ExitStack() as ctx:
        const = ctx.enter_context(tc.tile_pool(name="const", bufs=1))
        xfm_p = ctx.enter_context(tc.tile_pool(name="xfm", bufs=3))
        h_p = ctx.enter_context(tc.tile_pool(name="h", bufs=14))
        h1_p = ctx.enter_context(tc.tile_pool(name="h1", bufs=2))
        o_p = ctx.enter_context(tc.tile_pool(name="osb", bufs=1))
        cv_ps = ctx.enter_context(tc.tile_pool(name="cv_ps", bufs=4, space="PSUM"))
        f1_ps = ctx.enter_context(tc.tile_pool(name="f1_ps", bufs=2, space="PSUM"))
        f2_ps = ctx.enter_context(tc.tile_pool(name="f2_ps", bufs=2, space="PSUM"))

        kA = const.tile([112, 104], bf16, name="kA")
        nc.sync.dma_start(kA[:], kA_d)
        kB = const.tile([56, 104], bf16, name="kB")
        nc.sync.dma_start(kB[:], kB_d)
        kC = const.tile([112, 52], bf16, name="kC")
        nc.sync.dma_start(kC[:], kC_d)

        w1 = []
        offs = 0
        for b in range(7):
            m = 104 if b < 6 else 52
            t = const.tile([m, 128], bf16, tag=f"w1_{b}", name=f"w1_{b}")
            nc.sync.dma_start(t[:], w1_d[offs:offs + m, :])
            w1.append(t)
            offs += m
        w2 = const.tile([128, 10], bf16, name="w2")
        nc.sync.dma_start(w2[:], w2_d)
        b1 = const.tile([128, 1], f32, name="b1")
        nc.sync.dma_start(b1[:], b1_d)
        b2 = const.tile([10, 1], f32, name="b2")
        nc.sync.dma_start(b2[:], b2_d)

        y_sb = o_p.tile([10, b_core], f32, name="y_sb")

        # Two batch-tiles are processed as interleaved instruction streams:
        # consecutive PE matmuls belong to independent tiles (different PSUM
        # banks, independent deps) so fill/drain phases overlap.
        def emit_pair(its):
            xfms, hss, f1s, h1s = [], [], [], []
            for k, it in enumerate(its):
                xfm = xfm_p.tile([112, 7, BT], bf16, name="xfm", tag="xfm")
                src = x_d[:, :, it * BT:(it + 1) * BT].rearrange("g p b -> p g b")
                (nc.sync if it % 2 == 0 else nc.scalar).dma_start(xfm[:], src)
                xfms.append(xfm)
                hss.append([])

            for b in range(7):
                cvs = []
                if b < 6:
                    for k in range(len(its)):
                        cv = cv_ps.tile([104, BT], f32, name="cv", tag="cv")
                        nc.tensor.matmul(cv[:], kA[:], xfms[k][:, b, :],
                                         start=True, stop=False)
                        cvs.append(cv)
                    for k in range(len(its)):
                        nc.tensor.matmul(cvs[k][:], kB[:],
                                         xfms[k][0:56, b + 1, :],
                                         start=False, stop=True)
                else:
                    for k in range(len(its)):
                        cv = cv_ps.tile([52, BT], f32, name="cv6", tag="cv")
                        nc.tensor.matmul(cv[:], kC[:], xfms[k][:, 6, :],
                                         start=True, stop=True)
                        cvs.append(cv)
                for k in range(len(its)):
                    h = h_p.tile([104 if b < 6 else 52, BT], bf16, tag="h",
                                 name=f"h{b}_{k}")
                    if (b + k) % 2 == 0:
                        nc.vector.tensor_scalar_max(h[:], cvs[k][:], 0.0)
                    else:
                        nc.scalar.activation(h[:], cvs[k][:], AF.Relu)
                    hss[k].append(h)

            for k, it in enumerate(its):
                f1s.append(f1_ps.tile([128, BT], f32, name=f"f1_{k}", tag="f1"))
            for b in range(7):
                for k in range(len(its)):
                    nc.tensor.matmul(f1s[k][:], w1[b][:], hss[k][b][:],
                                     start=(b == 0), stop=(b == 6))
            for k, it in enumerate(its):
                h1 = h1_p.tile([128, BT], bf16, name=f"h1_{k}", tag="h1")
                nc.scalar.activation(h1[:], f1s[k][:], AF.Relu, bias=b1[:])
                h1s.append(h1)
            for k, it in enumerate(its):
                f2 = f2_ps.tile([10, BT], f32, name=f"f2_{k}", tag="f2")
                nc.tensor.matmul(f2[:], w2[:], h1s[k][:], start=True, stop=True)
                nc.scalar.activation(y_sb[:, it * BT:(it + 1) * BT], f2[:],
                                     AF.Identity, bias=b2[:])

        for it2 in range(0, nt, 2):
            emit_pair([it2, it2 + 1])

        # ---- single store at the very end (feature-major; host transposes)
        nc.sync.dma_start(y_d, y_sb[:])

    nc.compile()
    return nc


def _host_prep_v6(inputs):
    import ml_dtypes
    bf = ml_dtypes.bfloat16
    x = np.asarray(inputs["x"], dtype=np.float32)
    w = np.asarray(inputs["conv_w"], dtype=np.float32)
    fc1_w = np.asarray(inputs["fc1_w"], dtype=np.float32)
    fc1_b = np.asarray(inputs["fc1_b"], dtype=np.float32)
    fc2_w = np.asarray(inputs["fc2_w"], dtype=np.float32)
    fc2_b = np.asarray(inputs["fc2_b"], dtype=np.float32)

    B = x.shape[0]
    # feature-major: [7 row-groups, 112 pixels, B]
    xT = np.ascontiguousarray(x.astype(bf).reshape(B, 7, 112).transpose(1, 2, 0))

    kA = np.zeros((112, 104), np.float32)
    kB = np.zeros((56, 104), np.float32)
    kC = np.zeros((112, 52), np.float32)
    for oi in range(4):
        for oj in range(26):
            m = oi * 26 + oj
            for di in range(3):
                for dj in range(3):
                    ri, ci = oi + di, oj + dj
                    if ri < 4:
                        kA[ri * 28 + ci, m] = w[di, dj]
                    else:
                        kB[(ri - 4) * 28 + ci, m] = w[di, dj]
    for oi in range(2):
        for oj in range(26):
            m = oi * 26 + oj
            for di in range(3):
                for dj in range(3):
                    kC[(oi + di) * 28 + (oj + dj), m] = w[di, dj]

    consts = {
        "kA": kA.astype(bf),
        "kB": kB.astype(bf),
        "kC": kC.astype(bf),
        "w1": np.ascontiguousarray(fc1_w.T.astype(bf)),
        "b1": np.ascontiguousarray(fc1_b.reshape(128, 1)),
        "w2": np.ascontiguousarray(fc2_w.T.astype(bf)),
        "b2": np.ascontiguousarray(fc2_b.reshape(10, 1)),
    }
    in_maps = []
    for c in range(N_CORES):
        m = {"x": np.ascontiguousarray(xT[:, :, c * B_CORE:(c + 1) * B_CORE])}
        m.update(consts)
        in_maps.append(m)
    return in_maps


# ---------------------------------------------------------------------------
# v8: misaligned 128-out conv groups (11 conv MMs), dense fc1 (6 MMs), fc2 (1)
# = 18 matmuls per 512-batch tile.  Feature chunks are DMA'd pre-gathered and
# zero-padded so every stationary is [128, 128] (FWL-eligible) and every
# moving operand is a contiguous [128, 512] slice.  Software-pipelined skew:
# PE stream per tile = [fc2(t-2)] [conv(t) x11] [fc1(t-1) x6].
# ---------------------------------------------------------------------------

# chunk -> (pixel start, valid size); 2 chunks per 128-out group, 1 for the
# final 36-out group.  Receptive rows per group: g covers out rows
# [o//26 .. (o+m-1)//26 + 2] -> pixel span split into <=128-partition chunks.
V8_CHUNKS = [(0, 128), (128, 68), (112, 128), (240, 96), (252, 128), (380, 96),
             (392, 128), (520, 96), (532, 128), (660, 96), (672, 112)]
V8_CHUNK_GROUP = [0, 0, 1, 1, 2, 2, 3, 3, 4, 4, 5]
V8_GROUP_OUT = [(0, 128), (128, 128), (256, 128), (384, 128), (512, 128),
                (640, 36)]
V8_NCH = 11
V8_WORDS = V8_NCH * BT  # 5632 moving words per tile per partition


def _build_module_v8(b_core=B_CORE, n_cores=N_CORES, prefetch=3):
    import concourse.bass as bass
    import concourse.tile as tile
    from concourse import bacc, mybir

    f32 = mybir.dt.float32
    bf16 = mybir.dt.bfloat16
    AF = mybir.ActivationFunctionType
    nt = b_core // BT

    nc = bacc.Bacc("TRN2", target_bir_lowering=False, debug=False,
                   num_devices=n_cores)

    x_d = nc.dram_tensor("x", [nt, 128, V8_WORDS], bf16,
                         kind="ExternalInput").ap()
    # all bf16 weights in one blob: 11 conv chunks + 6 fc1 chunks ([128,128]
    # each) + fc2 [128,10] concatenated along free dim
    wb_d = nc.dram_tensor("wblob", [128, 17 * 128 + 10], bf16,
                          kind="ExternalInput").ap()
    bias_d = nc.dram_tensor("biases", [128, 2], f32, kind="ExternalInput").ap()
    y_d = nc.dram_tensor("y", [10, b_core], f32, kind="ExternalOutput").ap()

    with tile.TileContext(nc) as tc, ExitStack() as ctx:
        const = ctx.enter_context(tc.tile_pool(name="const", bufs=1))
        xg_p = ctx.enter_context(tc.tile_pool(name="xg", bufs=prefetch + 1))
        h_p = ctx.enter_context(tc.tile_pool(name="h", bufs=12))
        h1_p = ctx.enter_context(tc.tile_pool(name="h1", bufs=2))
        o_p = ctx.enter_context(tc.tile_pool(name="osb", bufs=1))
        cv_ps = ctx.enter_context(tc.tile_pool(name="cv_ps", bufs=5, space="PSUM"))
        f1_ps = ctx.enter_context(tc.tile_pool(name="f1_ps", bufs=2, space="PSUM"))
        f2_ps = ctx.enter_context(tc.tile_pool(name="f2_ps", bufs=1, space="PSUM"))

        wblob = const.tile([128, 17 * 128 + 10], bf16, name="wblob")
        nc.sync.dma_start(wblob[:], wb_d)
        biases = const.tile([128, 2], f32, name="biases")
        nc.sync.dma_start(biases[:], bias_d)
        wc = [wblob[:, c * 128:(c + 1) * 128] for c in range(V8_NCH)]
        w1 = [wblob[:, (V8_NCH + c) * 128:(V8_NCH + c + 1) * 128]
              for c in range(6)]
        w2 = wblob[:, 17 * 128:17 * 128 + 10]
        b1 = biases[:, 0:1]
        b2 = biases[0:10, 1:2]

        y_sb = o_p.tile([10, b_core], f32, name="y_sb")

        xgs = []

        def dma_x(t):
            xg = xg_p.tile([128, V8_WORDS], bf16, tag="xg", name=f"xg{t}")
            nc.sync.dma_start(xg[:], x_d[t])
            xgs.append(xg)

        hss = {}   # t -> list of 6 h tiles
        h1s = {}   # t -> h1 tile

        def emit_conv(t):
            xg = xgs.pop(0)
            hs = []
            for g in range(6):
                cv = cv_ps.tile([128, BT], f32, tag="cv", name=f"cv{g}")
                c0 = 2 * g
                if g < 5:
                    nc.tensor.matmul(cv[:], wc[c0], xg[:, c0 * BT:(c0 + 1) * BT],
                                     start=True, stop=False)
                    nc.tensor.matmul(cv[:], wc[c0 + 1],
                                     xg[:, (c0 + 1) * BT:(c0 + 2) * BT],
                                     start=False, stop=True)
                else:
                    nc.tensor.matmul(cv[:], wc[10], xg[:, 10 * BT:11 * BT],
                                     start=True, stop=True)
                h = h_p.tile([128, BT], bf16, tag="h", name=f"h{g}")
                if g % 2 == 0:
                    nc.vector.tensor_scalar_max(h[:], cv[:], 0.0)
                else:
                    nc.scalar.activation(h[:], cv[:], AF.Relu)
                hs.append(h)
            hss[t] = hs

        def emit_fc1(t):
            hs = hss.pop(t)
            f1 = f1_ps.tile([128, BT], f32, tag="f1", name="f1")
            for c in range(6):
                nc.tensor.matmul(f1[:], w1[c], hs[c][:],
                                 start=(c == 0), stop=(c == 5))
            h1 = h1_p.tile([128, BT], bf16, tag="h1", name="h1")
            nc.scalar.activation(h1[:], f1[:], AF.Relu, bias=b1)
            h1s[t] = h1

        def emit_fc2(t):
            h1 = h1s.pop(t)
            f2 = f2_ps.tile([10, BT], f32, tag="f2", name="f2")
            nc.tensor.matmul(f2[:], w2, h1[:], start=True, stop=True)
            nc.vector.tensor_scalar_add(y_sb[:, t * BT:(t + 1) * BT], f2[:], b2)

        for t in range(prefetch):
            dma_x(t)
        for t in range(nt):
            if t + prefetch < nt:
                dma_x(t + prefetch)
            if t >= 2:
                emit_fc2(t - 2)
            emit_conv(t)
            if t >= 1:
                emit_fc1(t - 1)
            # stream out finished quarter of y while compute continues
            if t % 4 == 3 and t >= 7:
                q = t // 4 - 1  # quarter fully written: tiles 4q..4q+3
                nc.sync.dma_start(y_d[:, q * 4 * BT:(q + 1) * 4 * BT],
                                  y_sb[:, q * 4 * BT:(q + 1) * 4 * BT])
        emit_fc1(nt - 1)
        emit_fc2(nt - 2)
        emit_fc2(nt - 1)
        nc.sync.dma_start(y_d[:, 12 * BT:], y_sb[:, 12 * BT:])

    nc.compile()
    return nc


def _host_prep_v8(inputs):
    import ml_dtypes
    bf = ml_dtypes.bfloat16
    x = np.asarray(inputs["x"], dtype=np.float32)
    w = np.asarray(inputs["conv_w"], dtype=np.float32)
    fc1_w = np.asarray(inputs["fc1_w"], dtype=np.float32)
    fc1_b = np.asarray(inputs["fc1_b"], dtype=np.float32)
    fc2_w = np.asarray(inputs["fc2_w"], dtype=np.float32)
    fc2_b = np.asarray(inputs["fc2_b"], dtype=np.float32)
    B = x.shape[0]
    nt = B_CORE // BT

    # gather pixels into zero-padded 128-partition chunks
    idx = np.full(V8_NCH * 128, 784, np.int64)
    for c, (s, sz) in enumerate(V8_CHUNKS):
        idx[c * 128:c * 128 + sz] = s + np.arange(sz)
    xb = np.concatenate([x.astype(bf), np.zeros((B, 1), bf)], axis=1)[:, idx]
    # [B, 11*128] -> [core, tile, partition, chunk, sample]
    xb = xb.reshape(N_CORES, nt, BT, V8_NCH, 128).transpose(0, 1, 4, 3, 2)
    xb = np.ascontiguousarray(xb).reshape(N_CORES, nt, 128, V8_WORDS)

    # conv band stationaries [11, 128, 128]
    wcs = np.zeros((V8_NCH, 128, 128), np.float32)
    p = np.arange(128)
    j = np.arange(128)
    for c, (start, size) in enumerate(V8_CHUNKS):
        g = V8_CHUNK_GROUP[c]
        ostart, osz = V8_GROUP_OUT[g]
        px = start + p
        o = ostart + j
        di = (px // 28)[:, None] - (o // 26)[None, :]
        dj = (px % 28)[:, None] - (o % 26)[None, :]
        m = ((di >= 0) & (di < 3) & (dj >= 0) & (dj < 3)
             & (p < size)[:, None] & (j < osz)[None, :])
        wcs[c][m] = w[np.clip(di, 0, 2), np.clip(dj, 0, 2)][m]

    # fc1 chunks [6, 128, 128] (rows grouped to match h layout, zero-padded)
    w1g = np.zeros((6, 128, 128), np.float32)
    for g, (os_, sz) in enumerate(V8_GROUP_OUT):
        w1g[g, :sz, :] = fc1_w.T[os_:os_ + sz, :]

    wblob = np.concatenate(
        [wcs.transpose(1, 0, 2).reshape(128, V8_NCH * 128),
         w1g.transpose(1, 0, 2).reshape(128, 6 * 128),
         fc2_w.T], axis=1).astype(bf)
    biases = np.zeros((128, 2), np.float32)
    biases[:, 0] = fc1_b
    biases[:10, 1] = fc2_b

    in_maps = []
    for c in range(N_CORES):
        in_maps.append({"x": xb[c], "wblob": wblob, "biases": biases})
    return in_maps


# ---------------------------------------------------------------------------
# v9: like v8 but x is stored as 8 overlapping 128-pixel columns (no zero
# padding, 1024 words/sample vs 1408) chosen so every conv group's receptive
# span is covered by 2 columns (1 for the last).  Stationaries zero out taps
# outside each MM's assigned pixel set (disjoint across a group's two MMs).
# Consts go on the scalar DMA queue (overlaps the first x DMA on sync), and
# 8 warmup matmuls on the weight blob flip the PE HAM clock gate to 8/8
# while the first x tile is still in flight.
# ---------------------------------------------------------------------------

V9_COLS = [0, 112, 220, 348, 392, 500, 628, 656]
V9_GROUP_COLS = [(0, 1), (1, 2), (2, 3), (4, 5), (5, 6), (7,)]
V9_NCOL = 8
V9_WORDS = V9_NCOL * BT  # 4096


def _build_module_v9(b_core=B_CORE, n_cores=N_CORES, prefetch=3, warmup=10):
    import concourse.bass as bass
    import concourse.tile as tile
    from concourse import bacc, mybir

    f32 = mybir.dt.float32
    bf16 = mybir.dt.bfloat16
    AF = mybir.ActivationFunctionType
    nt = b_core // BT
    nmm = 11  # conv matmuls per tile

    nc = bacc.Bacc("TRN2", target_bir_lowering=False, debug=False,
                   num_devices=n_cores)

    x_d = nc.dram_tensor("x", [nt, 128, V9_WORDS], bf16,
                         kind="ExternalInput").ap()
    wb_d = nc.dram_tensor("wblob", [128, (nmm + 6) * 128 + 10], bf16,
                          kind="ExternalInput").ap()
    bias_d = nc.dram_tensor("biases", [128, 2], f32, kind="ExternalInput").ap()
    y_d = nc.dram_tensor("y", [10, b_core], f32, kind="ExternalOutput").ap()

    with tile.TileContext(nc) as tc, ExitStack() as ctx:
        const = ctx.enter_context(tc.tile_pool(name="const", bufs=1))
        xg_p = ctx.enter_context(tc.tile_pool(name="xg", bufs=prefetch + 1))
        h_p = ctx.enter_context(tc.tile_pool(name="h", bufs=12))
        h1_p = ctx.enter_context(tc.tile_pool(name="h1", bufs=2))
        o_p = ctx.enter_context(tc.tile_pool(name="osb", bufs=1))
        cv_ps = ctx.enter_context(tc.tile_pool(name="cv_ps", bufs=5, space="PSUM"))
        f1_ps = ctx.enter_context(tc.tile_pool(name="f1_ps", bufs=2, space="PSUM"))
        f2_ps = ctx.enter_context(tc.tile_pool(name="f2_ps", bufs=1, space="PSUM"))

        # warm up the PE HAM clock gate on an on-chip zero tile (no DMA dep)
        warm_src = const.tile([128, BT], bf16, name="warm_src")
        nc.gpsimd.memset(warm_src[:], 0)
        for _ in range(warmup):
            wm = f1_ps.tile([128, BT], f32, tag="f1", name="warm")
            nc.tensor.matmul(wm[:], warm_src[:, 0:128], warm_src[:],
                             start=True, stop=True)

        wblob = const.tile([128, (nmm + 6) * 128 + 10], bf16, name="wblob")
        # conv weights first so the first conv matmul's dep is small
        nc.scalar.dma_start(wblob[:, :nmm * 128], wb_d[:, :nmm * 128])
        nc.scalar.dma_start(wblob[:, nmm * 128:], wb_d[:, nmm * 128:])
        biases = const.tile([128, 2], f32, name="biases")
        nc.scalar.dma_start(biases[:], bias_d)
        wc = [wblob[:, c * 128:(c + 1) * 128] for c in range(nmm)]
        w1 = [wblob[:, (nmm + c) * 128:(nmm + c + 1) * 128] for c in range(6)]
        w2 = wblob[:, (nmm + 6) * 128:(nmm + 6) * 128 + 10]
        b1 = biases[:, 0:1]
        b2 = biases[0:10, 1:2]

        y_sb = o_p.tile([10, b_core], f32, name="y_sb")

        xgs = []

        def dma_x(t, split=False):
            xg = xg_p.tile([128, V9_WORDS], bf16, tag="xg", name=f"xg{t}")
            if split:
                # conv g0 needs cols 0-1, g1 needs 1-2; rest arrive second
                for lo, hi in ((0, 3), (3, 8)):
                    nc.sync.dma_start(xg[:, lo * BT:hi * BT],
                                      x_d[t][:, lo * BT:hi * BT])
            else:
                nc.sync.dma_start(xg[:], x_d[t])
            xgs.append(xg)

        hss = {}
        h1s = {}

        def emit_conv(t):
            xg = xgs.pop(0)
            mi = 0
            hs = []
            for g in range(6):
                cols = V9_GROUP_COLS[g]
                pool = cv_ps if g < 5 else f2_ps
                cv = pool.tile([128, BT], f32, tag="f2" if g == 5 else "cv",
                               name=f"cv{g}")
                for k, col in enumerate(cols):
                    nc.tensor.matmul(cv[:], wc[mi],
                                     xg[:, col * BT:(col + 1) * BT],
                                     start=(k == 0), stop=(k == len(cols) - 1))
                    mi += 1
                h = h_p.tile([128, BT], bf16, tag="h", name=f"h{g}")
                if g % 2 == 0:
                    nc.vector.tensor_scalar_max(h[:], cv[:], 0.0)
                else:
                    nc.scalar.activation(h[:], cv[:], AF.Relu)
                hs.append(h)
            hss[t] = hs

        def emit_fc1(t):
            hs = hss.pop(t)
            f1 = f1_ps.tile([128, BT], f32, tag="f1", name="f1")
            for c in range(6):
                nc.tensor.matmul(f1[:], w1[c], hs[c][:],
                                 start=(c == 0), stop=(c == 5))
            h1 = h1_p.tile([128, BT], bf16, tag="h1", name="h1")
            nc.scalar.activation(h1[:], f1[:], AF.Relu, bias=b1)
            h1s[t] = h1

        def emit_fc2(t):
            h1 = h1s.pop(t)
            f2 = f2_ps.tile([128, BT], f32, tag="f2", name="f2")
            nc.tensor.matmul(f2[0:10, :], w2, h1[:], start=True, stop=True)
            nc.vector.tensor_scalar_add(y_sb[:, t * BT:(t + 1) * BT],
                                        f2[0:10, :], b2)

        for t in range(prefetch):
            dma_x(t, split=(t == 0))
        for t in range(nt):
            if t + prefetch < nt:
                dma_x(t + prefetch)
            if t >= 2:
                emit_fc2(t - 2)
            emit_conv(t)
            if t >= 1:
                emit_fc1(t - 1)
            if t % 4 == 3 and t >= 7:
                q = t // 4 - 1
                nc.sync.dma_start(y_d[:, q * 4 * BT:(q + 1) * 4 * BT],
                                  y_sb[:, q * 4 * BT:(q + 1) * 4 * BT])
        emit_fc1(nt - 1)
        emit_fc2(nt - 2)
        nc.sync.dma_start(y_d[:, 12 * BT:15 * BT], y_sb[:, 12 * BT:15 * BT])
        emit_fc2(nt - 1)
        nc.sync.dma_start(y_d[:, 15 * BT:], y_sb[:, 15 * BT:])

    nc.compile()
    return nc


def _host_prep_v9(inputs):
    import ml_dtypes
    bf = ml_dtypes.bfloat16
    x = np.asarray(inputs["x"], dtype=np.float32)
    w = np.asarray(inputs["conv_w"], dtype=np.float32)
    fc1_w = np.asarray(inputs["fc1_w"], dtype=np.float32)
    fc1_b = np.asarray(inputs["fc1_b"], dtype=np.float32)
    fc2_w = np.asarray(inputs["fc2_w"], dtype=np.float32)
    fc2_b = np.asarray(inputs["fc2_b"], dtype=np.float32)
    B = x.shape[0]
    nt = B_CORE // BT

    idx = np.concatenate([np.arange(a, a + 128) for a in V9_COLS])
    xb = x.astype(bf)[:, idx]
    xb = xb.reshape(N_CORES, nt, BT, V9_NCOL, 128).transpose(0, 1, 4, 3, 2)
    xb = np.ascontiguousarray(xb).reshape(N_CORES, nt, 128, V9_WORDS)

    # conv stationaries: per group, taps assigned to the first column that
    # contains the pixel (disjoint coverage across the group's matmuls)
    p = np.arange(128)
    j = np.arange(128)
    wcs = []
    for g, cols in enumerate(V9_GROUP_COLS):
        ostart, osz = V8_GROUP_OUT[g]
        covered_lo = None  # pixel range already handled by earlier col
        for col in cols:
            a = V9_COLS[col]
            px = a + p
            o = ostart + j
            di = (px // 28)[:, None] - (o // 26)[None, :]
            dj = (px % 28)[:, None] - (o % 26)[None, :]
            m = ((di >= 0) & (di < 3) & (dj >= 0) & (dj < 3)
                 & (j < osz)[None, :])
            if covered_lo is not None:
                lo, hi = covered_lo
                m &= ~((px >= lo) & (px < hi))[:, None]
            W = np.zeros((128, 128), np.float32)
            W[m] = w[np.clip(di, 0, 2), np.clip(dj, 0, 2)][m]
            wcs.append(W)
            covered_lo = (a, a + 128) if covered_lo is None else \
                (min(covered_lo[0], a), max(covered_lo[1], a + 128))
    wcs = np.stack(wcs)  # [11, 128, 128]

    w1g = np.zeros((6, 128, 128), np.float32)
    for g, (os_, sz) in enumerate(V8_GROUP_OUT):
        w1g[g, :sz, :] = fc1_w.T[os_:os_ + sz, :]

    wblob = np.concatenate(
        [wcs.transpose(1, 0, 2).reshape(128, 11 * 128),
         w1g.transpose(1, 0, 2).reshape(128, 6 * 128),
         fc2_w.T], axis=1).astype(bf)
    biases = np.zeros((128, 2), np.float32)
    biases[:, 0] = fc1_b
    biases[:10, 1] = fc2_b

    in_maps = []
    for c in range(N_CORES):
        in_maps.append({"x": xb[c], "wblob": wblob, "biases": biases})
    return in_maps


VERSION = 9


def run(inputs, trace=False, tmpdir=None, version=None):
    from concourse.bass_utils import run_bass_kernel_spmd

    version = VERSION if version is None else version
    key = f"nc{version}"
    builders = {9: _build_module_v9, 8: _build_module_v8, 6: _build_module_v6,
                4: _build_module_v4, 2: _build_module}
    preps = {9: _host_prep_v9, 8: _host_prep_v8, 6: _host_prep_v6,
             4: _host_prep_v4, 2: _host_prep}
    if key not in _cache:
        _cache[key] = builders[version]()
    nc = _cache[key]
    in_maps = preps[version](inputs)
    res = run_bass_kernel_spmd(nc, in_maps, list(range(N_CORES)), trace=trace,
                               tmpdir=tmpdir)
    out = np.concatenate([np.ascontiguousarray(r["y"].T) for r in res.results], axis=0)
    return out, res


def kernel(**inputs) -> np.ndarray:
    out, _ = run(inputs, trace=False)
    return out

